# revision 46
# baseline (speedup 1.0000x reference)
"""Bass/Tile Trainium2 kernel for a 2-layer dense multi-head GAT over a batch
of B=8 independent subgraphs (2048 nodes each, equal contiguous segments).

Sharding: one subgraph per NeuronCore (8 cores), parameters replicated.

Algorithm (per core / subgraph, per attention layer):
  scores are rank-1:  e_ij = leaky_relu(s1_i + s2_j),  s1 = h@a1, s2 = h@a2.
  exp(leaky_relu(t)) is separable through the sign mask M_ij = [s1_i+s2_j>=0]:
      p_ij = M_ij e^{s1_i} e^{s2_j} + (1-M_ij) e^{a s1_i} e^{a s2_j}
  so softmax(e) @ h needs NO N^2 exp work:
      num_i = g_i (M @ u)_i + (vtot - (M @ v))_i        u_j = e^{s2_j} [h_j|1]
      out_i = num_i[:64] / num_i[64]                    v_j = e^{a s2_j}[h_j|1]
                                                        g_i = e^{(1-a) s1_i}
  The N^2 work is one compare pass (mask tiles, exact in bf16) plus bf16 mask
  matmuls.  Mask generation is split across three engines: DVE/GpSimd emit
  0/1 masks (tensor_scalar is_ge); ACT emits +-1 sign masks (Sign activation
  with per-partition bias).  Sign chunks stream uv at half scale so that
  (2M-1)@(u/2) = M@u - utot_c/2, folded back via the PSUM seed row:
      A = [sum_c Mc@u | sum_c Mc@v] + seed,  seed = [S_u | -(vtotA + vtotS)]
  making num = g*A_lo - A_hi uniformly.  Layer-1 elu outputs are stored as
  elu+1 (= relu(x)+exp(min(x,0))), removing one N-wide op per tile; the +1
  shift is corrected exactly downstream (log_softmax is shift-invariant; the
  layer-2 projections get constant corrections c = colsum(wa2), r = colsum(W)).
"""

from contextlib import ExitStack

import numpy as np

import concourse.bass as bass
import concourse.tile as tile
from concourse import bacc, mybir
from concourse.masks import make_identity

FP = mybir.dt.float32
BF = mybir.dt.bfloat16
AF = mybir.ActivationFunctionType
OP = mybir.AluOpType

B = 8
N = 2048
D = 64
H = 4
ALPHA = 0.2
P = 128
NCH = N // P  # 16 chunks of 128 nodes
DEXT = D + 1  # h plus ones column

# mask-generation engine per j-chunk (per layer): ACT emits sign masks,
# GpSimd and DVE emit 0/1 masks.  Spread so no engine's chunks cluster.
ACT_CHUNKS = (3, 7, 11, 14)
POOL_CHUNKS = (5, 15)
# the output layer's masks cannot pre-generate (they need all 4 heads), so
# shift more of them onto the otherwise-idle ACT/GpSimd engines there
L2_ACT_CHUNKS = (1, 4, 7, 10, 13)
L2_POOL_CHUNKS = (2, 8, 14)
# double-buffered mask tags (generated a layer ahead); the rest single-buffer
MASK_BUFS2 = frozenset(range(5))


def _seed_groups(L):
    grpA = [c for c in range(NCH) if c not in L.act_chunks]  # 0/1 chunks
    grpS = list(L.act_chunks)  # sign chunks
    return grpA, grpS


class _Layer:
    """Holds one attention layer's prep tensors."""

    def __init__(self, tag):
        self.tag = tag
        self.act_chunks = L2_ACT_CHUNKS if tag == "o" else ACT_CHUNKS
        self.pool_chunks = L2_POOL_CHUNKS if tag == "o" else POOL_CHUNKS
        self.s12 = None      # [P, NCH, 2] fp32 (s1|s2 per node chunk)
        self.s2m = None      # callable jc -> [P,1] scalar AP for the mask op
        self.s1b = None      # [P, N] bf16, s1 replicated along free dim
        self.hext = None     # [P, NCH, DEXT] bf16, col D == 1.0
        self.g = None        # [P, NCH] fp32
        self.es2 = None      # [P, NCH] fp32 e^{s2}
        self.nes02 = None    # [P, NCH] fp32 e^{a s2}
        self.es2h = None     # halved versions (for sign chunks)
        self.nes02h = None
        self.uv = None       # [P, NCH, 2*DEXT] bf16
        self.sd_hi = None    # [1, 2*DEXT] bf16 seed row (hi)
        self.sd_res = None   # [1, 2*DEXT] bf16 seed row (residual)
        self.masks = {}      # jc -> [P, N] mask tile


def _emit_exps_and_uv(nc, pools, scratch, L, corr=None):
    """exps (ACT), halved copies, uv tiles (DVE), vt matmuls + seed rows."""
    const, prep, maskp, wide, small, psA, psaux, psvt = pools
    s12 = L.s12

    es2 = prep.tile([P, NCH], FP, tag="es2", name=f"es2_{L.tag}")
    nes02 = prep.tile([P, NCH], FP, tag="nes02", name=f"nes02_{L.tag}")
    g = prep.tile([P, NCH], FP, tag="g", name=f"g_{L.tag}")
    if corr is None:
        nc.scalar.activation(es2, s12[:, :, 1], AF.Exp)
        nc.scalar.activation(nes02, s12[:, :, 1], AF.Exp, scale=ALPHA)
        nc.scalar.activation(g, s12[:, :, 0], AF.Exp, scale=1.0 - ALPHA)
    else:
        # corrections for the xc+1 shift: corr = [cs | -c2 | -a*c2 | -(1-a)c1]
        nc.scalar.activation(es2, s12[:, :, 1], AF.Exp, bias=corr[:, 1:2])
        nc.scalar.activation(nes02, s12[:, :, 1], AF.Exp, scale=ALPHA,
                             bias=corr[:, 2:3])
        nc.scalar.activation(g, s12[:, :, 0], AF.Exp, scale=1.0 - ALPHA,
                             bias=corr[:, 3:4])
    es2h = prep.tile([P, NCH], FP, tag="es2h", name=f"es2h_{L.tag}")
    nc.vector.tensor_scalar(es2h, es2, 0.5, None, OP.mult)
    nes02h = prep.tile([P, NCH], FP, tag="nes02h", name=f"nes02h_{L.tag}")
    nc.vector.tensor_scalar(nes02h, nes02, 0.5, None, OP.mult)
    L.g, L.es2, L.nes02, L.es2h, L.nes02h = g, es2, nes02, es2h, nes02h

    # uv tiles (bf16, 4x DVE mode); sign chunks use the halved scalars.
    # DVE/GpSimd split the per-chunk ops to balance engine load.
    uv = prep.tile([P, NCH, 2 * DEXT], BF, tag="uv", name=f"uv_{L.tag}")
    for c in range(NCH):
        eu, ev = (es2h, nes02h) if c in L.act_chunks else (es2, nes02)
        eng = nc.vector if c % 2 == 0 else nc.gpsimd
        eng.tensor_scalar(uv[:, c, 0:DEXT], L.hext[:, c, :],
                          eu[:, c:c + 1], None, OP.mult)
        eng.tensor_scalar(uv[:, c, DEXT:], L.hext[:, c, :],
                          ev[:, c:c + 1], None, OP.mult)
    L.uv = uv


def _emit_vt_seed(nc, pools, scratch, L):
    """Column-total matmuls + seed rows.  Emitted late (hook q2) so the PE
    queue position comes after work whose inputs are long-ready -- the vt
    matmuls need ALL 16 uv chunks and would head-of-line-block the PE."""
    const, prep, maskp, wide, small, psA, psaux, psvt = pools
    uv = L.uv
    # column totals: vtA over 0/1 chunks (full scale), vtS over sign chunks
    # (half scale, exactly the S_c/2 sums the seed needs)
    grpA, grpS = _seed_groups(L)
    ones_col_bf = scratch["ones_col_bf"]
    vtA = psvt.tile([1, 2 * DEXT], FP, tag="vt", name=f"vtA_{L.tag}")
    vtS = psvt.tile([1, 2 * DEXT], FP, tag="vt", name=f"vtS_{L.tag}")
    for i, c in enumerate(grpA):
        nc.tensor.matmul(vtA, ones_col_bf, uv[:, c, :], start=(i == 0),
                         stop=(i == len(grpA) - 1))
    for i, c in enumerate(grpS):
        nc.tensor.matmul(vtS, ones_col_bf, uv[:, c, :], start=(i == 0),
                         stop=(i == len(grpS) - 1))

    # seed row sd = [vtS_u | -(vtA_v + vtS_v)] in fp32, then bf16 hi+res
    vts_sb = prep.tile([1, 2 * DEXT], FP, tag="vts", name=f"vts_{L.tag}")
    nc.vector.tensor_copy(vts_sb, vtS)
    sd = prep.tile([1, 2 * DEXT], FP, tag="sd", name=f"sd_{L.tag}")
    nc.vector.tensor_copy(sd[:, 0:DEXT], vts_sb[:, 0:DEXT])
    nc.vector.scalar_tensor_tensor(sd[:, DEXT:], vtA[:, DEXT:], -1.0,
                                   vts_sb[:, DEXT:], OP.mult, OP.subtract)
    sd_hi = prep.tile([1, 2 * DEXT], BF, tag="sdh", name=f"sdh_{L.tag}")
    nc.vector.tensor_copy(sd_hi, sd)
    sd_res = prep.tile([1, 2 * DEXT], BF, tag="sdr", name=f"sdr_{L.tag}")
    nc.vector.tensor_tensor(sd_res, sd, sd_hi, OP.subtract)
    L.sd_hi, L.sd_res = sd_hi, sd_res


def _emit_mask(nc, pools, L, jc):
    """One full-i mask tile [P, N] for j-chunk jc, on its assigned engine."""
    const, prep, maskp, wide, small, psA, psaux, psvt = pools
    bufs = 2 if jc in MASK_BUFS2 else 1
    mt = maskp.tile([P, N], BF, tag=f"m{jc}", name=f"m{jc}_{L.tag}",
                    bufs=bufs)
    s2s = L.s2m(jc)
    if jc in L.act_chunks:
        nc.scalar.activation(mt, L.s1b, AF.Sign, bias=s2s)
    elif jc in L.pool_chunks:
        nc.gpsimd.tensor_scalar(mt, L.s1b, s2s, 0.0, OP.add, OP.is_ge)
    else:
        nc.vector.tensor_scalar(mt, L.s1b, s2s, 0.0, OP.add, OP.is_ge)
    L.masks[jc] = mt


def _attention_quarters(nc, pools, scratch, L, out_cb, hooks=None):
    """Mask matmuls + epilogue for the 4 quarters of the i axis.

    hooks: optional dict q -> callable, emitted after quarter q's
    instructions (used to interleave the next layer's prep/mask emission).
    """
    const, prep, maskp, wide, small, psA, psaux, psvt = pools
    ones_row_bf = scratch["ones_row_bf"]

    nsum = wide.tile([P, NCH, DEXT], FP, tag="nsum", name=f"nsum_{L.tag}")
    rz = wide.tile([P, NCH], FP, tag="rz", name=f"rz_{L.tag}")
    onorm = wide.tile([P, NCH, D], FP, tag="onorm", name=f"onorm_{L.tag}")

    for q in range(4):
        # two accumulators share one PSUM bank -> bufs=4 spans two quarters,
        # so quarter q+1's matmuls never wait on quarter q's evacuation
        Ap = [psA.tile([P, 2, 2 * DEXT], FP, tag="A",
                       name=f"A{L.tag}_{q}_{pi}") for pi in range(2)]
        A = [Ap[il // 2][:, il % 2, :] for il in range(4)]
        for jc in range(NCH):
            mt = L.masks[jc]
            for il in range(4):
                sl = mt[:, q * 512 + il * P: q * 512 + (il + 1) * P]
                # start=True zeroes the whole 2KB bank, so only the pair's
                # first region sets it; the partner lands on zeroed PSUM
                nc.tensor.matmul(A[il], sl, L.uv[:, jc, :],
                                 start=(jc == 0 and il % 2 == 0), stop=False,
                                 skip_group_check=(il % 2 == 1))
        for il in range(4):
            nc.tensor.matmul(A[il], ones_row_bf, L.sd_hi, start=False,
                             stop=False)
            nc.tensor.matmul(A[il], ones_row_bf, L.sd_res, start=False,
                             stop=True)
        # epilogue: ACT evacuates each pair in one op, DVE combines
        # num = g*A_lo - A_hi from SBUF
        for pi in range(2):
            Asb = small.tile([P, 2, 2 * DEXT], FP, tag="Asb",
                             name=f"Asb_{L.tag}_{q}_{pi}")
            nc.scalar.copy(Asb, Ap[pi])
            for k in range(2):
                ic = q * 4 + pi * 2 + k
                nc.vector.scalar_tensor_tensor(nsum[:, ic, :],
                                               Asb[:, k, 0:DEXT],
                                               L.g[:, ic:ic + 1],
                                               Asb[:, k, DEXT:],
                                               OP.mult, OP.subtract)
        qs = slice(q * 4, (q + 1) * 4)
        nc.vector.reciprocal(rz[:, qs], nsum[:, qs, D])
        rzq = rz[:, qs]
        rz_b = bass.AP(tensor=rzq.tensor, offset=rzq.offset,
                       ap=[rzq.ap[0], rzq.ap[1], [0, D]])
        nc.vector.tensor_tensor(onorm[:, qs, :], nsum[:, qs, 0:D], rz_b,
                                OP.mult)
        out_cb(onorm, q)
        if hooks and q in hooks:
            hooks[q]()
    L.masks = {}


def _elu1_q(nc, wide, onorm, q, tag, dst, dst_sl, dve=False):
    """elu+1 over quarter q of onorm: dst = max(o,0) + exp(min(o,0))."""
    src = onorm[:, q * 4:(q + 1) * 4, :]
    eng = nc.vector if dve else nc.gpsimd
    m = wide.tile([P, 4, D], FP, tag="elu_m", name=f"elu_m{tag}_{q}")
    eng.tensor_scalar(m, src, 0.0, None, OP.min)
    e = wide.tile([P, 4, D], FP, tag="elu_e", name=f"elu_e{tag}_{q}")
    nc.scalar.activation(e, m, AF.Exp)
    r = wide.tile([P, 4, D], FP, tag="elu_r", name=f"elu_r{tag}_{q}")
    eng.tensor_scalar(r, src, 0.0, None, OP.max)
    eng.tensor_tensor(dst[dst_sl], r, e, OP.add)


def build_kernel():
    nc = bacc.Bacc("TRN2", target_bir_lowering=False, debug=False,
                   num_devices=B)

    x = nc.dram_tensor("x", [N, D], FP, kind="ExternalInput")
    W_heads = nc.dram_tensor("W_heads", [H, D, D], FP, kind="ExternalInput")
    a_heads = nc.dram_tensor("a_heads", [H, 2 * D], FP, kind="ExternalInput")
    W_out = nc.dram_tensor("W_out", [H * D, D], FP, kind="ExternalInput")
    a_out = nc.dram_tensor("a_out", [2 * D], FP, kind="ExternalInput")
    out = nc.dram_tensor("out", [N, D], FP, kind="ExternalOutput")

    with tile.TileContext(nc) as tc, ExitStack() as ctx:
        const = ctx.enter_context(tc.tile_pool(name="const", bufs=1))
        prep = ctx.enter_context(tc.tile_pool(name="prep", bufs=2))
        maskp = ctx.enter_context(tc.tile_pool(name="maskp", bufs=1))
        wide = ctx.enter_context(tc.tile_pool(name="wide", bufs=2))
        small = ctx.enter_context(tc.tile_pool(name="small", bufs=4))
        psA = ctx.enter_context(tc.tile_pool(name="psA", bufs=4, space="PSUM"))
        psaux = ctx.enter_context(tc.tile_pool(name="psaux", bufs=2,
                                               space="PSUM"))
        psvt = ctx.enter_context(tc.tile_pool(name="psvt", bufs=2,
                                              space="PSUM"))
        pools = (const, prep, maskp, wide, small, psA, psaux, psvt)

        ident = const.tile([P, P], FP)
        make_identity(nc, ident)
        ones128 = const.tile([P, P], FP)
        nc.vector.memset(ones128, 1.0)
        ones_col_bf = const.tile([P, 1], BF)
        nc.vector.memset(ones_col_bf, 1.0)
        ones_row_bf = const.tile([1, P], BF)
        nc.vector.memset(ones_row_bf, 1.0)
        scratch = {"ones128": ones128, "ones_col_bf": ones_col_bf,
                   "ones_row_bf": ones_row_bf}

        # ---- input DMAs: x in 4 pieces, params in NATURAL layouts only
        # (transposed layouts would be 4-byte-gather DMAs; transpose on PE).
        # Interleaved so HWDGE serializing 632ns/DMA doesn't delay the params
        # (which gate the wa/s12 chain) behind all of x.
        x_sb = const.tile([P, NCH, D], FP)
        x_r = x.rearrange("(c p) d -> p c d", p=P)
        Wh = const.tile([64, H, D], FP)
        a_sb = const.tile([64, H, 2], FP)
        Wo = const.tile([P, 2, D], FP)
        ao = const.tile([64, 2], FP)

        def xdma(r4):
            nc.sync.dma_start(out=x_sb[:, r4 * 4:(r4 + 1) * 4, :],
                              in_=x_r[:, r4 * 4:(r4 + 1) * 4, :])
        xdma(0)
        nc.sync.dma_start(out=Wh, in_=W_heads.rearrange("h k d -> k h d"))
        nc.sync.dma_start(out=a_sb,
                          in_=a_heads.rearrange("h (t k) -> k h t", t=2))
        xdma(1)
        nc.sync.dma_start(out=Wo, in_=W_out.rearrange("(c k) d -> k c d", k=P))
        nc.sync.dma_start(out=ao, in_=a_out.rearrange("(t k) -> k t", t=2))
        xdma(2)
        xdma(3)

        # ---- param transposes on PE + bf16 shadows ----
        WhT = const.tile([64, H, D], FP)
        for h in range(H):
            tp = psaux.tile([64, D], FP, tag="aux", name=f"whT{h}")
            nc.tensor.transpose(tp, Wh[:, h, :], ident[0:64, 0:64])
            nc.scalar.copy(WhT[:, h, :], tp)
        WoT = const.tile([64, 2, P], FP)
        for kc in range(2):
            tp = psaux.tile([64, P], FP, tag="aux", name=f"woT{kc}")
            nc.tensor.transpose(tp, Wo[:, kc, :], ident)
            nc.scalar.copy(WoT[:, kc, :], tp)
        Wh_bf = const.tile([64, H, D], BF)
        nc.vector.tensor_copy(Wh_bf, Wh)
        Wo_bf = const.tile([P, 2, D], BF)
        nc.vector.tensor_copy(Wo_bf, Wo)

        # ---- x transposes -> xT fp32 + bf16 shadow ----
        xT = const.tile([64, N], FP)
        for c in range(NCH):
            tp = psaux.tile([64, P], FP, tag="aux", name=f"xT{c}")
            nc.tensor.transpose(tp, x_sb[:, c, :], ident)
            if c % 2 == 0:
                nc.vector.tensor_copy(xT[:, c * P:(c + 1) * P], tp)
            else:
                nc.scalar.copy(xT[:, c * P:(c + 1) * P], tp)
        xT_bf = const.tile([64, N], BF)
        for r in range(4):
            eng = nc.vector if r % 2 == 0 else nc.gpsimd
            eng.tensor_copy(xT_bf[:, r * 512:(r + 1) * 512],
                            xT[:, r * 512:(r + 1) * 512])

        # all heads' wa = W_h @ [a1|a2] upfront
        wa_all = const.tile([64, H, 2], FP)
        for h in range(H):
            wap = psaux.tile([64, 2], FP, tag="aux", name=f"wap{h}")
            nc.tensor.matmul(wap, WhT[:, h, :], a_sb[:, h, :], start=True,
                             stop=True)
            nc.scalar.copy(wa_all[:, h, :], wap)

        # ================= layer 1: four heads =================
        xc01 = const.tile([P, NCH, 2, D], FP)
        xc23 = const.tile([P, NCH, 2, D], FP)

        def l1_prep(h):
            L = _Layer(f"h{h}")
            wa = wa_all[:, h, :]
            s12 = prep.tile([P, NCH, 2], FP, tag="s12", name=f"s12_{h}")
            for cg in range(4):
                sp = psaux.tile([P, 8], FP, tag="aux", name=f"sp{h}_{cg}")
                for k in range(4):
                    c = cg * 4 + k
                    nc.tensor.matmul(sp[:, 2 * k:2 * k + 2],
                                     xT[:, c * P:(c + 1) * P], wa,
                                     start=True, stop=True)
                nc.vector.tensor_copy(s12[:, cg * 4:(cg + 1) * 4, :], sp)
            L.s12 = s12
            L.s2m = lambda jc: s12[:, jc, 1:2]

            wa1b = prep.tile([64, P], BF, tag="wa1b", name=f"wa1b_{h}")
            nc.vector.tensor_scalar(wa1b, ones128[0:64, :], wa[:, 0:1], None,
                                    OP.mult)
            s1b = prep.tile([P, N], BF, tag="s1b", name=f"s1b_{h}")
            for r in range(4):
                ps = psaux.tile([P, 512], FP, tag="aux", name=f"s1bp{h}_{r}")
                nc.tensor.matmul(ps, wa1b, xT_bf[:, r * 512:(r + 1) * 512],
                                 start=True, stop=True)
                nc.scalar.copy(s1b[:, r * 512:(r + 1) * 512], ps)
            L.s1b = s1b

            hext = prep.tile([P, NCH, DEXT], BF, tag="hext", name=f"hext_{h}")
            nc.vector.memset(hext[:, :, D], 1.0)
            for c in range(NCH):
                hp = psaux.tile([P, D], FP, tag="aux", name=f"hp{h}_{c}")
                nc.tensor.matmul(hp, xT_bf[:, c * P:(c + 1) * P],
                                 Wh_bf[:, h, :], start=True, stop=True)
                nc.vector.tensor_copy(hext[:, c, 0:D], hp)
            L.hext = hext
            _emit_exps_and_uv(nc, pools, scratch, L)
            return L

        def l1_out(L, h):
            xc = xc01 if h < 2 else xc23

            def cb(onorm, q):
                _elu1_q(nc, wide, onorm, q, f"h{h}", xc,
                        np.s_[:, q * 4:(q + 1) * 4, h % 2, :])
            return cb

        # ---- xc transposes (emitted interleaved via hooks) ----
        xcT = const.tile([P, 2, N], FP)
        xcT_bf = const.tile([P, 2, N], BF)

        def xc_transpose(kc, c0, c1):
            xc = xc01 if kc == 0 else xc23
            for c in range(c0, c1):
                tp = psaux.tile([P, P], FP, tag="aux", name=f"xcT{kc}_{c}")
                nc.tensor.transpose(tp, xc[:, c, :, :], ident)
                if c % 2 == 0:
                    nc.vector.tensor_copy(xcT[:, kc, c * P:(c + 1) * P], tp)
                else:
                    nc.scalar.copy(xcT[:, kc, c * P:(c + 1) * P], tp)

        def xcbf_copy(kc, r0, r1):
            for r in range(r0, r1):
                eng = nc.gpsimd if r % 2 == 0 else nc.vector
                eng.tensor_copy(xcT_bf[:, kc, r * 512:(r + 1) * 512],
                                xcT[:, kc, r * 512:(r + 1) * 512])

        # ================= layer 2 prep =================
        def l2_prep_part1():
            """Everything that only needs parameters (+ c/r correction rows)."""
            st = {}
            wa2 = prep.tile([P, 2, 2], FP, tag="wa2")
            for kc in range(2):
                wap = psaux.tile([P, 2], FP, tag="aux", name=f"wap2_{kc}")
                nc.tensor.matmul(wap, WoT[:, kc, :], ao, start=True, stop=True)
                nc.scalar.copy(wa2[:, kc, :], wap)
            st["wa2"] = wa2
            # c = colsum(wa2) [1,2] -> broadcast [128,2]; r = colsum(W_out)
            c_ps = psvt.tile([1, 2], FP, tag="vt", name="c_ps")
            for kc in range(2):
                nc.tensor.matmul(c_ps, ones128[:, 0:1], wa2[:, kc, :],
                                 start=(kc == 0), stop=(kc == 1))
            c_sb = prep.tile([1, 2], FP, tag="c_sb")
            nc.vector.tensor_copy(c_sb, c_ps)
            cb_ps = psvt.tile([P, 2], FP, tag="vt", name="cb_ps")
            nc.tensor.matmul(cb_ps, ones128[0:1, :], c_sb, start=True,
                             stop=True)
            cb = prep.tile([P, 2], FP, tag="cb")
            nc.vector.tensor_copy(cb, cb_ps)
            corr = prep.tile([P, 4], FP, tag="corr")
            nc.vector.tensor_tensor(corr[:, 0:1], cb[:, 0:1], cb[:, 1:2],
                                    OP.add)
            nc.vector.tensor_scalar(corr[:, 1:2], cb[:, 1:2], -1.0, None,
                                    OP.mult)
            nc.vector.tensor_scalar(corr[:, 2:3], cb[:, 1:2], -ALPHA, None,
                                    OP.mult)
            nc.vector.tensor_scalar(corr[:, 3:4], cb[:, 0:1],
                                    -(1.0 - ALPHA), None, OP.mult)
            st["corr"] = corr
            r_ps = psvt.tile([1, D], FP, tag="vt", name="r_ps")
            for kc in range(2):
                nc.tensor.matmul(r_ps, ones_col_bf, Wo_bf[:, kc, :],
                                 start=(kc == 0), stop=(kc == 1))
            nr = prep.tile([1, D], BF, tag="nr")
            nc.vector.tensor_scalar(nr, r_ps, -1.0, None, OP.mult)
            st["nr"] = nr
            return st

        def l2_prep_part2(st):
            """Needs xcT/xcT_bf: s12_2, s2m, s1b_2, hext2, exps, uv, seeds."""
            L = _Layer("o")
            wa2, corr, nr = st["wa2"], st["corr"], st["nr"]
            s12 = prep.tile([P, NCH, 2], FP, tag="s12", name="s12_o")
            for cg in range(4):
                sp = psaux.tile([P, 8], FP, tag="aux", name=f"sp2_{cg}")
                for k in range(4):
                    c = cg * 4 + k
                    for kc in range(2):
                        nc.tensor.matmul(sp[:, 2 * k:2 * k + 2],
                                         xcT[:, kc, c * P:(c + 1) * P],
                                         wa2[:, kc, :],
                                         start=(kc == 0), stop=(kc == 1))
                nc.vector.tensor_copy(s12[:, cg * 4:(cg + 1) * 4, :], sp)
            L.s12 = s12
            # s2m = s2' - (c1+c2) so the masks compare the true s1+s2 >= 0
            s2m = prep.tile([P, NCH], FP, tag="s2m")
            nc.vector.tensor_scalar(s2m, s12[:, :, 1], corr[:, 0:1], None,
                                    OP.subtract)
            L.s2m = lambda jc: s2m[:, jc:jc + 1]

            wa1b2 = prep.tile([P, 2, P], BF, tag="wa1b2")
            for kc in range(2):
                nc.vector.tensor_scalar(wa1b2[:, kc, :], ones128,
                                        wa2[:, kc, 0:1], None, OP.mult)
            s1b = prep.tile([P, N], BF, tag="s1b", name="s1b_o")
            for r in range(4):
                ps = psaux.tile([P, 512], FP, tag="aux", name=f"s1bp_o{r}")
                for kc in range(2):
                    nc.tensor.matmul(ps, wa1b2[:, kc, :],
                                     xcT_bf[:, kc, r * 512:(r + 1) * 512],
                                     start=(kc == 0), stop=(kc == 1))
                nc.scalar.copy(s1b[:, r * 512:(r + 1) * 512], ps)
            L.s1b = s1b

            hext = prep.tile([P, NCH, DEXT], BF, tag="hext", name="hext_o")
            nc.vector.memset(hext[:, :, D], 1.0)
            for c in range(NCH):
                hp = psaux.tile([P, D], FP, tag="aux", name=f"hp_o{c}")
                for kc in range(2):
                    nc.tensor.matmul(hp, xcT_bf[:, kc, c * P:(c + 1) * P],
                                     Wo_bf[:, kc, :], start=(kc == 0),
                                     stop=False)
                nc.tensor.matmul(hp, ones_row_bf, nr, start=False, stop=True)
                nc.vector.tensor_copy(hext[:, c, 0:D], hp)
            L.hext = hext
            _emit_exps_and_uv(nc, pools, scratch, L, corr=corr)
            return L

        # ================= emission schedule =================
        Ls = [None] * 5  # heads 0..3 + output layer
        Ls[0] = l1_prep(0)
        _emit_vt_seed(nc, pools, scratch, Ls[0])
        for jc in range(NCH):
            _emit_mask(nc, pools, Ls[0], jc)

        l2st = {}

        def mk_hooks(h):
            # Interleave next layer's prep + mask emission into head h's
            # quarter stream.  Only double-buffered mask tags may be emitted
            # before head h's last quarter (single-buffered tags would be
            # overwritten before head h's q2/q3 matmuls are even emitted).
            def prep_next():
                if h + 1 <= 3:
                    Ls[h + 1] = l1_prep(h + 1)
                    _emit_vt_seed(nc, pools, scratch, Ls[h + 1])
                if h == 2:
                    xc_transpose(0, 12, NCH)

            def masks_a():
                if h + 1 <= 3:
                    for jc in sorted(MASK_BUFS2):
                        _emit_mask(nc, pools, Ls[h + 1], jc)
                if h == 1:
                    xc_transpose(0, 0, 8)
                if h == 2:
                    xcbf_copy(0, 0, 4)

            def masks_b():
                if h == 1:
                    xc_transpose(0, 8, 12)
                if h == 2:
                    l2st.update(l2_prep_part1())

            def masks_c():
                if h + 1 <= 3:
                    for jc in range(NCH):
                        if jc not in MASK_BUFS2:
                            _emit_mask(nc, pools, Ls[h + 1], jc)
            return {0: prep_next, 1: masks_a, 2: masks_b, 3: masks_c}

        for h in range(H):
            hooks = mk_hooks(h)
            if h == 3:
                base = hooks
                def h3_hooks(q, base=base):
                    def f():
                        if q in base:
                            base[q]()
                        if q >= 1:
                            xc_transpose(1, (q - 1) * 4, q * 4)
                    return f
                hooks = {q: h3_hooks(q) for q in range(4)}
            _attention_quarters(nc, pools, scratch, Ls[h], l1_out(Ls[h], h),
                                hooks)

        xc_transpose(1, 12, NCH)
        xcbf_copy(1, 0, 4)
        Ls[4] = l2_prep_part2(l2st)
        _emit_vt_seed(nc, pools, scratch, Ls[4])
        for jc in range(NCH):
            _emit_mask(nc, pools, Ls[4], jc)

        # ================= layer 2 attention + log_softmax =================
        o2_all = const.tile([P, NCH, D], FP)
        esum = const.tile([P, NCH], FP)
        lse = const.tile([P, NCH], FP)
        out_r = out.rearrange("(c p) d -> p c d", p=P)

        def finish_half(hf):
            # Ln batched per half: Exp and Ln live in different default act
            # tables, so finer batching would thrash LoadActFuncSet
            sl = slice(hf * 8, (hf + 1) * 8)
            nc.scalar.activation(lse[:, sl], esum[:, sl], AF.Ln)
            for sq in range(2):
                qs = slice(hf * 8 + sq * 4, hf * 8 + (sq + 1) * 4)
                lseh = lse[:, qs]
                lse_b = bass.AP(tensor=lseh.tensor, offset=lseh.offset,
                                ap=[lseh.ap[0], lseh.ap[1], [0, D]])
                nc.vector.tensor_tensor(o2_all[:, qs, :], o2_all[:, qs, :],
                                        lse_b, OP.subtract)
                nc.sync.dma_start(out=out_r[:, qs, :], in_=o2_all[:, qs, :])

        def l2_out(onorm, q):
            qs = slice(q * 4, (q + 1) * 4)
            _elu1_q(nc, wide, onorm, q, "o", o2_all, np.s_[:, qs, :])
            escr = wide.tile([P, 4, D], FP, tag="escr", name=f"escr{q}")
            nc.scalar.activation(escr, o2_all[:, qs, :], AF.Exp)
            nc.vector.tensor_reduce(esum[:, qs], escr,
                                    mybir.AxisListType.X, OP.add)
            if q == 1:
                finish_half(0)
            elif q == 3:
                finish_half(1)

        _attention_quarters(nc, pools, scratch, Ls[4], l2_out)

    nc.compile()
    return nc


_NC_CACHE = {}


def _make_runner(nc):
    """Build a cached sharded executable (run_bass_kernel_spmd re-traces
    jax.jit on every call; this jits once and reuses)."""
    import jax
    from jax.sharding import Mesh, PartitionSpec
    try:
        from jax.experimental.shard_map import shard_map
    except ImportError:
        from jax.shard_map import shard_map
    import concourse.mybir as mb
    from concourse import bass2jax

    bass2jax.install_neuronx_cc_hook()

    part_name = nc.partition_id_tensor.name if nc.partition_id_tensor else None
    in_names, out_names, out_avals = [], [], []
    for alloc in nc.m.functions[0].allocations:
        if not isinstance(alloc, mb.MemoryLocationSet):
            continue
        name = alloc.memorylocations[0].name
        if alloc.kind == "ExternalInput":
            if name != part_name:
                in_names.append(name)
        elif alloc.kind == "ExternalOutput":
            out_names.append(name)
            out_avals.append(jax.core.ShapedArray(
                tuple(alloc.tensor_shape), mb.dt.np(alloc.dtype)))
    n_params = len(in_names)
    all_names = in_names + out_names
    if part_name is not None:
        all_names = all_names + [part_name]

    def _body(*args):
        operands = list(args)
        if part_name is not None:
            operands.append(bass2jax.partition_id_tensor())
        return tuple(bass2jax._bass_exec_p.bind(
            *operands, out_avals=tuple(out_avals), in_names=tuple(all_names),
            out_names=tuple(out_names), lowering_input_output_aliases=(),
            sim_require_finite=True, sim_require_nnan=True, nc=nc))

    devices = jax.devices()[:B]
    mesh = Mesh(np.asarray(devices), ("core",))
    n_outs = len(out_names)
    sharded = jax.jit(
        shard_map(_body, mesh=mesh,
                  in_specs=(PartitionSpec("core"),) * (n_params + n_outs),
                  out_specs=(PartitionSpec("core"),) * n_outs,
                  check_rep=False),
        donate_argnums=tuple(range(n_params, n_params + n_outs)),
        keep_unused=True)

    def run(in_maps):
        concat_in = [
            np.concatenate([np.asarray(in_maps[c][nm])[None] for c in range(B)],
                           axis=0).reshape(B * in_maps[0][nm].shape[0],
                                           *in_maps[0][nm].shape[1:])
            for nm in in_names
        ]
        concat_zeros = [
            np.zeros((B * av.shape[0], *av.shape[1:]), av.dtype)
            for av in out_avals
        ]
        out_arrs = sharded(*concat_in, *concat_zeros)
        return [
            {nm: np.asarray(out_arrs[i]).reshape(B, *out_avals[i].shape)[c]
             for i, nm in enumerate(out_names)}
            for c in range(B)
        ]

    return run


def kernel(**inputs):
    h_states = np.ascontiguousarray(np.asarray(inputs["h_states"], dtype=np.float32))
    W_heads = np.ascontiguousarray(np.asarray(inputs["W_heads"], dtype=np.float32))
    a_heads = np.ascontiguousarray(np.asarray(inputs["a_heads"], dtype=np.float32))
    W_out = np.ascontiguousarray(np.asarray(inputs["W_out"], dtype=np.float32))
    a_out = np.ascontiguousarray(np.asarray(inputs["a_out"], dtype=np.float32))

    if "nc" not in _NC_CACHE:
        _NC_CACHE["nc"] = build_kernel()
        _NC_CACHE["run"] = _make_runner(_NC_CACHE["nc"])

    xs = h_states.reshape(B, N, D)
    in_maps = [
        {"x": xs[c], "W_heads": W_heads, "a_heads": a_heads,
         "W_out": W_out, "a_out": a_out}
        for c in range(B)
    ]
    results = _NC_CACHE["run"](in_maps)
    return np.concatenate([results[c]["out"] for c in range(B)], axis=0)


if __name__ == "__main__":
    rng = np.random.default_rng(0)
    inputs = {
        "h_states": rng.standard_normal((B * N, D)).astype(np.float32),
        "W_heads": rng.standard_normal((H, D, D)).astype(np.float32) * 0.18,
        "a_heads": rng.standard_normal((H, 2 * D)).astype(np.float32) * 0.18,
        "W_out": rng.standard_normal((H * D, D)).astype(np.float32) * 0.09,
        "a_out": rng.standard_normal((2 * D,)).astype(np.float32) * 0.18,
        "seq_start_end": (np.arange(B, dtype=np.int32)[:, None] * N
                          + np.array([0, N], dtype=np.int32)[None, :]),
    }
    got = kernel(**inputs)
    print("kernel output", got.shape, got.dtype)


# revision 47
# speedup vs baseline: 1.0038x; 1.0038x over previous
"""Bass/Tile Trainium2 kernel for a 2-layer dense multi-head GAT over a batch
of B=8 independent subgraphs (2048 nodes each, equal contiguous segments).

Sharding: one subgraph per NeuronCore (8 cores), parameters replicated.

Algorithm (per core / subgraph, per attention layer):
  scores are rank-1:  e_ij = leaky_relu(s1_i + s2_j),  s1 = h@a1, s2 = h@a2.
  exp(leaky_relu(t)) is separable through the sign mask M_ij = [s1_i+s2_j>=0]:
      p_ij = M_ij e^{s1_i} e^{s2_j} + (1-M_ij) e^{a s1_i} e^{a s2_j}
  so softmax(e) @ h needs NO N^2 exp work:
      num_i = g_i (M @ u)_i + (vtot - (M @ v))_i        u_j = e^{s2_j} [h_j|1]
      out_i = num_i[:64] / num_i[64]                    v_j = e^{a s2_j}[h_j|1]
                                                        g_i = e^{(1-a) s1_i}
  The N^2 work is one compare pass (mask tiles, exact in bf16) plus bf16 mask
  matmuls.  Mask generation is split across three engines: DVE/GpSimd emit
  0/1 masks (tensor_scalar is_ge); ACT emits +-1 sign masks (Sign activation
  with per-partition bias).  Sign chunks stream uv at half scale so that
  (2M-1)@(u/2) = M@u - utot_c/2, folded back via the PSUM seed row:
      A = [sum_c Mc@u | sum_c Mc@v] + seed,  seed = [S_u | -(vtotA + vtotS)]
  making num = g*A_lo - A_hi uniformly.  Layer-1 elu outputs are stored as
  elu+1 (= relu(x)+exp(min(x,0))), removing one N-wide op per tile; the +1
  shift is corrected exactly downstream (log_softmax is shift-invariant; the
  layer-2 projections get constant corrections c = colsum(wa2), r = colsum(W)).
"""

from contextlib import ExitStack

import numpy as np

import concourse.bass as bass
import concourse.tile as tile
from concourse import bacc, mybir
from concourse.masks import make_identity

FP = mybir.dt.float32
BF = mybir.dt.bfloat16
AF = mybir.ActivationFunctionType
OP = mybir.AluOpType

B = 8
N = 2048
D = 64
H = 4
ALPHA = 0.2
P = 128
NCH = N // P  # 16 chunks of 128 nodes
DEXT = D + 1  # h plus ones column

# mask-generation engine per j-chunk (per layer): ACT emits sign masks,
# GpSimd and DVE emit 0/1 masks.  Spread so no engine's chunks cluster.
ACT_CHUNKS = (3, 7, 11, 14)
POOL_CHUNKS = (5, 15)
# the output layer's masks cannot pre-generate (they need all 4 heads), so
# shift more of them onto the otherwise-idle ACT/GpSimd engines there
L2_ACT_CHUNKS = ACT_CHUNKS
L2_POOL_CHUNKS = POOL_CHUNKS
# double-buffered mask tags (generated a layer ahead); the rest single-buffer
MASK_BUFS2 = frozenset(range(5))


def _seed_groups(L):
    grpA = [c for c in range(NCH) if c not in L.act_chunks]  # 0/1 chunks
    grpS = list(L.act_chunks)  # sign chunks
    return grpA, grpS


class _Layer:
    """Holds one attention layer's prep tensors."""

    def __init__(self, tag):
        self.tag = tag
        self.act_chunks = L2_ACT_CHUNKS if tag == "o" else ACT_CHUNKS
        self.pool_chunks = L2_POOL_CHUNKS if tag == "o" else POOL_CHUNKS
        self.s12 = None      # [P, NCH, 2] fp32 (s1|s2 per node chunk)
        self.s2m = None      # callable jc -> [P,1] scalar AP for the mask op
        self.s1b = None      # [P, N] bf16, s1 replicated along free dim
        self.hext = None     # [P, NCH, DEXT] bf16, col D == 1.0
        self.g = None        # [P, NCH] fp32
        self.es2 = None      # [P, NCH] fp32 e^{s2}
        self.nes02 = None    # [P, NCH] fp32 e^{a s2}
        self.es2h = None     # halved versions (for sign chunks)
        self.nes02h = None
        self.uv = None       # [P, NCH, 2*DEXT] bf16
        self.sd_hi = None    # [1, 2*DEXT] bf16 seed row (hi)
        self.sd_res = None   # [1, 2*DEXT] bf16 seed row (residual)
        self.masks = {}      # jc -> [P, N] mask tile


def _emit_exps_and_uv(nc, pools, scratch, L, corr=None):
    """exps (ACT), halved copies, uv tiles (DVE), vt matmuls + seed rows."""
    const, prep, maskp, wide, small, psA, psaux, psvt = pools
    s12 = L.s12

    es2 = prep.tile([P, NCH], FP, tag="es2", name=f"es2_{L.tag}")
    nes02 = prep.tile([P, NCH], FP, tag="nes02", name=f"nes02_{L.tag}")
    g = prep.tile([P, NCH], FP, tag="g", name=f"g_{L.tag}")
    if corr is None:
        nc.scalar.activation(es2, s12[:, :, 1], AF.Exp)
        nc.scalar.activation(nes02, s12[:, :, 1], AF.Exp, scale=ALPHA)
        nc.scalar.activation(g, s12[:, :, 0], AF.Exp, scale=1.0 - ALPHA)
    else:
        # corrections for the xc+1 shift: corr = [cs | -c2 | -a*c2 | -(1-a)c1]
        nc.scalar.activation(es2, s12[:, :, 1], AF.Exp, bias=corr[:, 1:2])
        nc.scalar.activation(nes02, s12[:, :, 1], AF.Exp, scale=ALPHA,
                             bias=corr[:, 2:3])
        nc.scalar.activation(g, s12[:, :, 0], AF.Exp, scale=1.0 - ALPHA,
                             bias=corr[:, 3:4])
    es2h = prep.tile([P, NCH], FP, tag="es2h", name=f"es2h_{L.tag}")
    nc.vector.tensor_scalar(es2h, es2, 0.5, None, OP.mult)
    nes02h = prep.tile([P, NCH], FP, tag="nes02h", name=f"nes02h_{L.tag}")
    nc.vector.tensor_scalar(nes02h, nes02, 0.5, None, OP.mult)
    L.g, L.es2, L.nes02, L.es2h, L.nes02h = g, es2, nes02, es2h, nes02h

    # uv tiles (bf16, 4x DVE mode); sign chunks use the halved scalars.
    # DVE/GpSimd split the per-chunk ops to balance engine load.
    uv = prep.tile([P, NCH, 2 * DEXT], BF, tag="uv", name=f"uv_{L.tag}")
    for c in range(NCH):
        eu, ev = (es2h, nes02h) if c in L.act_chunks else (es2, nes02)
        eng = nc.vector if c % 2 == 0 else nc.gpsimd
        eng.tensor_scalar(uv[:, c, 0:DEXT], L.hext[:, c, :],
                          eu[:, c:c + 1], None, OP.mult)
        eng.tensor_scalar(uv[:, c, DEXT:], L.hext[:, c, :],
                          ev[:, c:c + 1], None, OP.mult)
    L.uv = uv


def _emit_vt_seed(nc, pools, scratch, L):
    """Column-total matmuls + seed rows.  Emitted late (hook q2) so the PE
    queue position comes after work whose inputs are long-ready -- the vt
    matmuls need ALL 16 uv chunks and would head-of-line-block the PE."""
    const, prep, maskp, wide, small, psA, psaux, psvt = pools
    uv = L.uv
    # column totals: vtA over 0/1 chunks (full scale), vtS over sign chunks
    # (half scale, exactly the S_c/2 sums the seed needs)
    grpA, grpS = _seed_groups(L)
    ones_col_bf = scratch["ones_col_bf"]
    vtA = psvt.tile([1, 2 * DEXT], FP, tag="vt", name=f"vtA_{L.tag}")
    vtS = psvt.tile([1, 2 * DEXT], FP, tag="vt", name=f"vtS_{L.tag}")
    for i, c in enumerate(grpA):
        nc.tensor.matmul(vtA, ones_col_bf, uv[:, c, :], start=(i == 0),
                         stop=(i == len(grpA) - 1))
    for i, c in enumerate(grpS):
        nc.tensor.matmul(vtS, ones_col_bf, uv[:, c, :], start=(i == 0),
                         stop=(i == len(grpS) - 1))

    # seed row sd = [vtS_u | -(vtA_v + vtS_v)] in fp32, then bf16 hi+res
    vts_sb = prep.tile([1, 2 * DEXT], FP, tag="vts", name=f"vts_{L.tag}")
    nc.vector.tensor_copy(vts_sb, vtS)
    sd = prep.tile([1, 2 * DEXT], FP, tag="sd", name=f"sd_{L.tag}")
    nc.vector.tensor_copy(sd[:, 0:DEXT], vts_sb[:, 0:DEXT])
    nc.vector.scalar_tensor_tensor(sd[:, DEXT:], vtA[:, DEXT:], -1.0,
                                   vts_sb[:, DEXT:], OP.mult, OP.subtract)
    sd_hi = prep.tile([1, 2 * DEXT], BF, tag="sdh", name=f"sdh_{L.tag}")
    nc.vector.tensor_copy(sd_hi, sd)
    sd_res = prep.tile([1, 2 * DEXT], BF, tag="sdr", name=f"sdr_{L.tag}")
    nc.vector.tensor_tensor(sd_res, sd, sd_hi, OP.subtract)
    L.sd_hi, L.sd_res = sd_hi, sd_res


def _emit_mask(nc, pools, L, jc):
    """One full-i mask tile [P, N] for j-chunk jc, on its assigned engine."""
    const, prep, maskp, wide, small, psA, psaux, psvt = pools
    bufs = 2 if jc in MASK_BUFS2 else 1
    mt = maskp.tile([P, N], BF, tag=f"m{jc}", name=f"m{jc}_{L.tag}",
                    bufs=bufs)
    s2s = L.s2m(jc)
    if jc in L.act_chunks:
        nc.scalar.activation(mt, L.s1b, AF.Sign, bias=s2s)
    elif jc in L.pool_chunks:
        nc.gpsimd.tensor_scalar(mt, L.s1b, s2s, 0.0, OP.add, OP.is_ge)
    else:
        nc.vector.tensor_scalar(mt, L.s1b, s2s, 0.0, OP.add, OP.is_ge)
    L.masks[jc] = mt


def _attention_quarters(nc, pools, scratch, L, out_cb, hooks=None):
    """Mask matmuls + epilogue for the 4 quarters of the i axis.

    hooks: optional dict q -> callable, emitted after quarter q's
    instructions (used to interleave the next layer's prep/mask emission).
    """
    const, prep, maskp, wide, small, psA, psaux, psvt = pools
    ones_row_bf = scratch["ones_row_bf"]

    nsum = wide.tile([P, NCH, DEXT], FP, tag="nsum", name=f"nsum_{L.tag}")
    rz = wide.tile([P, NCH], FP, tag="rz", name=f"rz_{L.tag}")
    onorm = wide.tile([P, NCH, D], FP, tag="onorm", name=f"onorm_{L.tag}")

    for q in range(4):
        # two accumulators share one PSUM bank -> bufs=4 spans two quarters,
        # so quarter q+1's matmuls never wait on quarter q's evacuation
        Ap = [psA.tile([P, 2, 2 * DEXT], FP, tag="A",
                       name=f"A{L.tag}_{q}_{pi}") for pi in range(2)]
        A = [Ap[il // 2][:, il % 2, :] for il in range(4)]
        for jc in range(NCH):
            mt = L.masks[jc]
            for il in range(4):
                sl = mt[:, q * 512 + il * P: q * 512 + (il + 1) * P]
                # start=True zeroes the whole 2KB bank, so only the pair's
                # first region sets it; the partner lands on zeroed PSUM
                nc.tensor.matmul(A[il], sl, L.uv[:, jc, :],
                                 start=(jc == 0 and il % 2 == 0), stop=False,
                                 skip_group_check=(il % 2 == 1))
        for il in range(4):
            nc.tensor.matmul(A[il], ones_row_bf, L.sd_hi, start=False,
                             stop=False)
            nc.tensor.matmul(A[il], ones_row_bf, L.sd_res, start=False,
                             stop=True)
        # epilogue: ACT evacuates each pair in one op, DVE combines
        # num = g*A_lo - A_hi from SBUF
        for pi in range(2):
            Asb = small.tile([P, 2, 2 * DEXT], FP, tag="Asb",
                             name=f"Asb_{L.tag}_{q}_{pi}")
            nc.scalar.copy(Asb, Ap[pi])
            for k in range(2):
                ic = q * 4 + pi * 2 + k
                nc.vector.scalar_tensor_tensor(nsum[:, ic, :],
                                               Asb[:, k, 0:DEXT],
                                               L.g[:, ic:ic + 1],
                                               Asb[:, k, DEXT:],
                                               OP.mult, OP.subtract)
        qs = slice(q * 4, (q + 1) * 4)
        nc.vector.reciprocal(rz[:, qs], nsum[:, qs, D])
        rzq = rz[:, qs]
        rz_b = bass.AP(tensor=rzq.tensor, offset=rzq.offset,
                       ap=[rzq.ap[0], rzq.ap[1], [0, D]])
        nc.vector.tensor_tensor(onorm[:, qs, :], nsum[:, qs, 0:D], rz_b,
                                OP.mult)
        out_cb(onorm, q)
        if hooks and q in hooks:
            hooks[q]()
    L.masks = {}


def _elu1_q(nc, wide, onorm, q, tag, dst, dst_sl, dve=False):
    """elu+1 over quarter q of onorm: dst = max(o,0) + exp(min(o,0))."""
    src = onorm[:, q * 4:(q + 1) * 4, :]
    eng = nc.vector if dve else nc.gpsimd
    m = wide.tile([P, 4, D], FP, tag="elu_m", name=f"elu_m{tag}_{q}")
    eng.tensor_scalar(m, src, 0.0, None, OP.min)
    e = wide.tile([P, 4, D], FP, tag="elu_e", name=f"elu_e{tag}_{q}")
    nc.scalar.activation(e, m, AF.Exp)
    r = wide.tile([P, 4, D], FP, tag="elu_r", name=f"elu_r{tag}_{q}")
    eng.tensor_scalar(r, src, 0.0, None, OP.max)
    eng.tensor_tensor(dst[dst_sl], r, e, OP.add)


def build_kernel():
    nc = bacc.Bacc("TRN2", target_bir_lowering=False, debug=False,
                   num_devices=B)

    x = nc.dram_tensor("x", [N, D], FP, kind="ExternalInput")
    W_heads = nc.dram_tensor("W_heads", [H, D, D], FP, kind="ExternalInput")
    a_heads = nc.dram_tensor("a_heads", [H, 2 * D], FP, kind="ExternalInput")
    W_out = nc.dram_tensor("W_out", [H * D, D], FP, kind="ExternalInput")
    a_out = nc.dram_tensor("a_out", [2 * D], FP, kind="ExternalInput")
    out = nc.dram_tensor("out", [N, D], FP, kind="ExternalOutput")

    with tile.TileContext(nc) as tc, ExitStack() as ctx:
        const = ctx.enter_context(tc.tile_pool(name="const", bufs=1))
        prep = ctx.enter_context(tc.tile_pool(name="prep", bufs=2))
        maskp = ctx.enter_context(tc.tile_pool(name="maskp", bufs=1))
        wide = ctx.enter_context(tc.tile_pool(name="wide", bufs=2))
        small = ctx.enter_context(tc.tile_pool(name="small", bufs=4))
        psA = ctx.enter_context(tc.tile_pool(name="psA", bufs=4, space="PSUM"))
        psaux = ctx.enter_context(tc.tile_pool(name="psaux", bufs=2,
                                               space="PSUM"))
        psvt = ctx.enter_context(tc.tile_pool(name="psvt", bufs=2,
                                              space="PSUM"))
        pools = (const, prep, maskp, wide, small, psA, psaux, psvt)

        ident = const.tile([P, P], FP)
        make_identity(nc, ident)
        ones128 = const.tile([P, P], FP)
        nc.vector.memset(ones128, 1.0)
        ones_col_bf = const.tile([P, 1], BF)
        nc.vector.memset(ones_col_bf, 1.0)
        ones_row_bf = const.tile([1, P], BF)
        nc.vector.memset(ones_row_bf, 1.0)
        scratch = {"ones128": ones128, "ones_col_bf": ones_col_bf,
                   "ones_row_bf": ones_row_bf}

        # ---- input DMAs: x in 4 pieces, params in NATURAL layouts only
        # (transposed layouts would be 4-byte-gather DMAs; transpose on PE).
        # Interleaved so HWDGE serializing 632ns/DMA doesn't delay the params
        # (which gate the wa/s12 chain) behind all of x.
        x_sb = const.tile([P, NCH, D], FP)
        x_r = x.rearrange("(c p) d -> p c d", p=P)
        Wh = const.tile([64, H, D], FP)
        a_sb = const.tile([64, H, 2], FP)
        Wo = const.tile([P, 2, D], FP)
        ao = const.tile([64, 2], FP)

        def xdma(r4):
            nc.sync.dma_start(out=x_sb[:, r4 * 4:(r4 + 1) * 4, :],
                              in_=x_r[:, r4 * 4:(r4 + 1) * 4, :])
        xdma(0)
        nc.sync.dma_start(out=Wh, in_=W_heads.rearrange("h k d -> k h d"))
        nc.sync.dma_start(out=a_sb,
                          in_=a_heads.rearrange("h (t k) -> k h t", t=2))
        xdma(1)
        nc.sync.dma_start(out=Wo, in_=W_out.rearrange("(c k) d -> k c d", k=P))
        nc.sync.dma_start(out=ao, in_=a_out.rearrange("(t k) -> k t", t=2))
        xdma(2)
        xdma(3)

        # ---- param transposes on PE + bf16 shadows ----
        WhT = const.tile([64, H, D], FP)
        for h in range(H):
            tp = psaux.tile([64, D], FP, tag="aux", name=f"whT{h}")
            nc.tensor.transpose(tp, Wh[:, h, :], ident[0:64, 0:64])
            nc.scalar.copy(WhT[:, h, :], tp)
        WoT = const.tile([64, 2, P], FP)
        for kc in range(2):
            tp = psaux.tile([64, P], FP, tag="aux", name=f"woT{kc}")
            nc.tensor.transpose(tp, Wo[:, kc, :], ident)
            nc.scalar.copy(WoT[:, kc, :], tp)
        Wh_bf = const.tile([64, H, D], BF)
        nc.vector.tensor_copy(Wh_bf, Wh)
        Wo_bf = const.tile([P, 2, D], BF)
        nc.vector.tensor_copy(Wo_bf, Wo)

        # ---- x transposes -> xT fp32 + bf16 shadow ----
        xT = const.tile([64, N], FP)
        for c in range(NCH):
            tp = psaux.tile([64, P], FP, tag="aux", name=f"xT{c}")
            nc.tensor.transpose(tp, x_sb[:, c, :], ident)
            if c % 2 == 0:
                nc.vector.tensor_copy(xT[:, c * P:(c + 1) * P], tp)
            else:
                nc.scalar.copy(xT[:, c * P:(c + 1) * P], tp)
        xT_bf = const.tile([64, N], BF)
        for r in range(4):
            eng = nc.vector if r % 2 == 0 else nc.gpsimd
            eng.tensor_copy(xT_bf[:, r * 512:(r + 1) * 512],
                            xT[:, r * 512:(r + 1) * 512])

        # all heads' wa = W_h @ [a1|a2] upfront
        wa_all = const.tile([64, H, 2], FP)
        for h in range(H):
            wap = psaux.tile([64, 2], FP, tag="aux", name=f"wap{h}")
            nc.tensor.matmul(wap, WhT[:, h, :], a_sb[:, h, :], start=True,
                             stop=True)
            nc.scalar.copy(wa_all[:, h, :], wap)

        # ================= layer 1: four heads =================
        xc01 = const.tile([P, NCH, 2, D], FP)
        xc23 = const.tile([P, NCH, 2, D], FP)

        def l1_prep(h):
            L = _Layer(f"h{h}")
            wa = wa_all[:, h, :]
            s12 = prep.tile([P, NCH, 2], FP, tag="s12", name=f"s12_{h}")
            for cg in range(4):
                sp = psaux.tile([P, 8], FP, tag="aux", name=f"sp{h}_{cg}")
                for k in range(4):
                    c = cg * 4 + k
                    nc.tensor.matmul(sp[:, 2 * k:2 * k + 2],
                                     xT[:, c * P:(c + 1) * P], wa,
                                     start=True, stop=True)
                nc.vector.tensor_copy(s12[:, cg * 4:(cg + 1) * 4, :], sp)
            L.s12 = s12
            L.s2m = lambda jc: s12[:, jc, 1:2]

            wa1b = prep.tile([64, P], BF, tag="wa1b", name=f"wa1b_{h}")
            nc.vector.tensor_scalar(wa1b, ones128[0:64, :], wa[:, 0:1], None,
                                    OP.mult)
            s1b = prep.tile([P, N], BF, tag="s1b", name=f"s1b_{h}")
            for r in range(4):
                ps = psaux.tile([P, 512], FP, tag="aux", name=f"s1bp{h}_{r}")
                nc.tensor.matmul(ps, wa1b, xT_bf[:, r * 512:(r + 1) * 512],
                                 start=True, stop=True)
                nc.scalar.copy(s1b[:, r * 512:(r + 1) * 512], ps)
            L.s1b = s1b

            hext = prep.tile([P, NCH, DEXT], BF, tag="hext", name=f"hext_{h}")
            nc.vector.memset(hext[:, :, D], 1.0)
            for c in range(NCH):
                hp = psaux.tile([P, D], FP, tag="aux", name=f"hp{h}_{c}")
                nc.tensor.matmul(hp, xT_bf[:, c * P:(c + 1) * P],
                                 Wh_bf[:, h, :], start=True, stop=True)
                nc.vector.tensor_copy(hext[:, c, 0:D], hp)
            L.hext = hext
            _emit_exps_and_uv(nc, pools, scratch, L)
            return L

        def l1_out(L, h):
            xc = xc01 if h < 2 else xc23

            def cb(onorm, q):
                _elu1_q(nc, wide, onorm, q, f"h{h}", xc,
                        np.s_[:, q * 4:(q + 1) * 4, h % 2, :])
            return cb

        # ---- xc transposes (emitted interleaved via hooks) ----
        xcT = const.tile([P, 2, N], FP)
        xcT_bf = const.tile([P, 2, N], BF)

        def xc_transpose(kc, c0, c1):
            xc = xc01 if kc == 0 else xc23
            for c in range(c0, c1):
                tp = psaux.tile([P, P], FP, tag="aux", name=f"xcT{kc}_{c}")
                nc.tensor.transpose(tp, xc[:, c, :, :], ident)
                if c % 2 == 0:
                    nc.vector.tensor_copy(xcT[:, kc, c * P:(c + 1) * P], tp)
                else:
                    nc.scalar.copy(xcT[:, kc, c * P:(c + 1) * P], tp)

        def xcbf_copy(kc, r0, r1):
            for r in range(r0, r1):
                eng = nc.gpsimd if r % 2 == 0 else nc.vector
                eng.tensor_copy(xcT_bf[:, kc, r * 512:(r + 1) * 512],
                                xcT[:, kc, r * 512:(r + 1) * 512])

        # ================= layer 2 prep =================
        def l2_prep_part1():
            """Everything that only needs parameters (+ c/r correction rows)."""
            st = {}
            wa2 = prep.tile([P, 2, 2], FP, tag="wa2")
            for kc in range(2):
                wap = psaux.tile([P, 2], FP, tag="aux", name=f"wap2_{kc}")
                nc.tensor.matmul(wap, WoT[:, kc, :], ao, start=True, stop=True)
                nc.scalar.copy(wa2[:, kc, :], wap)
            st["wa2"] = wa2
            # c = colsum(wa2) [1,2] -> broadcast [128,2]; r = colsum(W_out)
            c_ps = psvt.tile([1, 2], FP, tag="vt", name="c_ps")
            for kc in range(2):
                nc.tensor.matmul(c_ps, ones128[:, 0:1], wa2[:, kc, :],
                                 start=(kc == 0), stop=(kc == 1))
            c_sb = prep.tile([1, 2], FP, tag="c_sb")
            nc.vector.tensor_copy(c_sb, c_ps)
            cb_ps = psvt.tile([P, 2], FP, tag="vt", name="cb_ps")
            nc.tensor.matmul(cb_ps, ones128[0:1, :], c_sb, start=True,
                             stop=True)
            cb = prep.tile([P, 2], FP, tag="cb")
            nc.vector.tensor_copy(cb, cb_ps)
            corr = prep.tile([P, 4], FP, tag="corr")
            nc.vector.tensor_tensor(corr[:, 0:1], cb[:, 0:1], cb[:, 1:2],
                                    OP.add)
            nc.vector.tensor_scalar(corr[:, 1:2], cb[:, 1:2], -1.0, None,
                                    OP.mult)
            nc.vector.tensor_scalar(corr[:, 2:3], cb[:, 1:2], -ALPHA, None,
                                    OP.mult)
            nc.vector.tensor_scalar(corr[:, 3:4], cb[:, 0:1],
                                    -(1.0 - ALPHA), None, OP.mult)
            st["corr"] = corr
            r_ps = psvt.tile([1, D], FP, tag="vt", name="r_ps")
            for kc in range(2):
                nc.tensor.matmul(r_ps, ones_col_bf, Wo_bf[:, kc, :],
                                 start=(kc == 0), stop=(kc == 1))
            nr = prep.tile([1, D], BF, tag="nr")
            nc.vector.tensor_scalar(nr, r_ps, -1.0, None, OP.mult)
            st["nr"] = nr
            return st

        def l2_prep_part2(st):
            """Needs xcT/xcT_bf: s12_2, s2m, s1b_2, hext2, exps, uv, seeds."""
            L = _Layer("o")
            wa2, corr, nr = st["wa2"], st["corr"], st["nr"]
            s12 = prep.tile([P, NCH, 2], FP, tag="s12", name="s12_o")
            for cg in range(4):
                sp = psaux.tile([P, 8], FP, tag="aux", name=f"sp2_{cg}")
                for k in range(4):
                    c = cg * 4 + k
                    for kc in range(2):
                        nc.tensor.matmul(sp[:, 2 * k:2 * k + 2],
                                         xcT[:, kc, c * P:(c + 1) * P],
                                         wa2[:, kc, :],
                                         start=(kc == 0), stop=(kc == 1))
                nc.vector.tensor_copy(s12[:, cg * 4:(cg + 1) * 4, :], sp)
            L.s12 = s12
            # s2m = s2' - (c1+c2) so the masks compare the true s1+s2 >= 0
            s2m = prep.tile([P, NCH], FP, tag="s2m")
            nc.vector.tensor_scalar(s2m, s12[:, :, 1], corr[:, 0:1], None,
                                    OP.subtract)
            L.s2m = lambda jc: s2m[:, jc:jc + 1]

            wa1b2 = prep.tile([P, 2, P], BF, tag="wa1b2")
            for kc in range(2):
                nc.vector.tensor_scalar(wa1b2[:, kc, :], ones128,
                                        wa2[:, kc, 0:1], None, OP.mult)
            s1b = prep.tile([P, N], BF, tag="s1b", name="s1b_o")
            for r in range(4):
                ps = psaux.tile([P, 512], FP, tag="aux", name=f"s1bp_o{r}")
                for kc in range(2):
                    nc.tensor.matmul(ps, wa1b2[:, kc, :],
                                     xcT_bf[:, kc, r * 512:(r + 1) * 512],
                                     start=(kc == 0), stop=(kc == 1))
                nc.scalar.copy(s1b[:, r * 512:(r + 1) * 512], ps)
            L.s1b = s1b

            hext = prep.tile([P, NCH, DEXT], BF, tag="hext", name="hext_o")
            nc.vector.memset(hext[:, :, D], 1.0)
            for c in range(NCH):
                hp = psaux.tile([P, D], FP, tag="aux", name=f"hp_o{c}")
                for kc in range(2):
                    nc.tensor.matmul(hp, xcT_bf[:, kc, c * P:(c + 1) * P],
                                     Wo_bf[:, kc, :], start=(kc == 0),
                                     stop=False)
                nc.tensor.matmul(hp, ones_row_bf, nr, start=False, stop=True)
                nc.vector.tensor_copy(hext[:, c, 0:D], hp)
            L.hext = hext
            _emit_exps_and_uv(nc, pools, scratch, L, corr=corr)
            return L

        # ================= emission schedule =================
        Ls = [None] * 5  # heads 0..3 + output layer
        Ls[0] = l1_prep(0)
        _emit_vt_seed(nc, pools, scratch, Ls[0])
        for jc in range(NCH):
            _emit_mask(nc, pools, Ls[0], jc)

        l2st = {}

        def mk_hooks(h):
            # Interleave next layer's prep + mask emission into head h's
            # quarter stream.  Only double-buffered mask tags may be emitted
            # before head h's last quarter (single-buffered tags would be
            # overwritten before head h's q2/q3 matmuls are even emitted).
            def prep_next():
                if h + 1 <= 3:
                    Ls[h + 1] = l1_prep(h + 1)
                    _emit_vt_seed(nc, pools, scratch, Ls[h + 1])
                if h == 2:
                    xc_transpose(0, 12, NCH)

            def masks_a():
                if h + 1 <= 3:
                    for jc in sorted(MASK_BUFS2):
                        _emit_mask(nc, pools, Ls[h + 1], jc)
                if h == 1:
                    xc_transpose(0, 0, 8)
                if h == 2:
                    xcbf_copy(0, 0, 4)

            def masks_b():
                if h == 1:
                    xc_transpose(0, 8, 12)
                if h == 2:
                    l2st.update(l2_prep_part1())

            def masks_c():
                if h + 1 <= 3:
                    for jc in range(NCH):
                        if jc not in MASK_BUFS2:
                            _emit_mask(nc, pools, Ls[h + 1], jc)
            return {0: prep_next, 1: masks_a, 2: masks_b, 3: masks_c}

        for h in range(H):
            hooks = mk_hooks(h)
            if h == 3:
                base = hooks
                def h3_hooks(q, base=base):
                    def f():
                        if q in base:
                            base[q]()
                        if q >= 1:
                            xc_transpose(1, (q - 1) * 4, q * 4)
                    return f
                hooks = {q: h3_hooks(q) for q in range(4)}
            _attention_quarters(nc, pools, scratch, Ls[h], l1_out(Ls[h], h),
                                hooks)

        xc_transpose(1, 12, NCH)
        xcbf_copy(1, 0, 4)
        Ls[4] = l2_prep_part2(l2st)
        _emit_vt_seed(nc, pools, scratch, Ls[4])
        for jc in range(NCH):
            _emit_mask(nc, pools, Ls[4], jc)

        # ================= layer 2 attention + log_softmax =================
        o2_all = const.tile([P, NCH, D], FP)
        esum = const.tile([P, NCH], FP)
        lse = const.tile([P, NCH], FP)
        out_r = out.rearrange("(c p) d -> p c d", p=P)

        def finish_half(hf):
            # Ln batched per half: Exp and Ln live in different default act
            # tables, so finer batching would thrash LoadActFuncSet
            sl = slice(hf * 8, (hf + 1) * 8)
            nc.scalar.activation(lse[:, sl], esum[:, sl], AF.Ln)
            for sq in range(2):
                qs = slice(hf * 8 + sq * 4, hf * 8 + (sq + 1) * 4)
                lseh = lse[:, qs]
                lse_b = bass.AP(tensor=lseh.tensor, offset=lseh.offset,
                                ap=[lseh.ap[0], lseh.ap[1], [0, D]])
                nc.vector.tensor_tensor(o2_all[:, qs, :], o2_all[:, qs, :],
                                        lse_b, OP.subtract)
                nc.sync.dma_start(out=out_r[:, qs, :], in_=o2_all[:, qs, :])

        def l2_out(onorm, q):
            qs = slice(q * 4, (q + 1) * 4)
            _elu1_q(nc, wide, onorm, q, "o", o2_all, np.s_[:, qs, :])
            escr = wide.tile([P, 4, D], FP, tag="escr", name=f"escr{q}")
            nc.scalar.activation(escr, o2_all[:, qs, :], AF.Exp)
            nc.vector.tensor_reduce(esum[:, qs], escr,
                                    mybir.AxisListType.X, OP.add)
            if q == 1:
                finish_half(0)
            elif q == 3:
                finish_half(1)

        _attention_quarters(nc, pools, scratch, Ls[4], l2_out)

    nc.compile()
    return nc


_NC_CACHE = {}


def _make_runner(nc):
    """Build a cached sharded executable (run_bass_kernel_spmd re-traces
    jax.jit on every call; this jits once and reuses)."""
    import jax
    from jax.sharding import Mesh, PartitionSpec
    try:
        from jax.experimental.shard_map import shard_map
    except ImportError:
        from jax.shard_map import shard_map
    import concourse.mybir as mb
    from concourse import bass2jax

    bass2jax.install_neuronx_cc_hook()

    part_name = nc.partition_id_tensor.name if nc.partition_id_tensor else None
    in_names, out_names, out_avals = [], [], []
    for alloc in nc.m.functions[0].allocations:
        if not isinstance(alloc, mb.MemoryLocationSet):
            continue
        name = alloc.memorylocations[0].name
        if alloc.kind == "ExternalInput":
            if name != part_name:
                in_names.append(name)
        elif alloc.kind == "ExternalOutput":
            out_names.append(name)
            out_avals.append(jax.core.ShapedArray(
                tuple(alloc.tensor_shape), mb.dt.np(alloc.dtype)))
    n_params = len(in_names)
    all_names = in_names + out_names
    if part_name is not None:
        all_names = all_names + [part_name]

    def _body(*args):
        operands = list(args)
        if part_name is not None:
            operands.append(bass2jax.partition_id_tensor())
        return tuple(bass2jax._bass_exec_p.bind(
            *operands, out_avals=tuple(out_avals), in_names=tuple(all_names),
            out_names=tuple(out_names), lowering_input_output_aliases=(),
            sim_require_finite=True, sim_require_nnan=True, nc=nc))

    devices = jax.devices()[:B]
    mesh = Mesh(np.asarray(devices), ("core",))
    n_outs = len(out_names)
    sharded = jax.jit(
        shard_map(_body, mesh=mesh,
                  in_specs=(PartitionSpec("core"),) * (n_params + n_outs),
                  out_specs=(PartitionSpec("core"),) * n_outs,
                  check_rep=False),
        donate_argnums=tuple(range(n_params, n_params + n_outs)),
        keep_unused=True)

    def run(in_maps):
        concat_in = [
            np.concatenate([np.asarray(in_maps[c][nm])[None] for c in range(B)],
                           axis=0).reshape(B * in_maps[0][nm].shape[0],
                                           *in_maps[0][nm].shape[1:])
            for nm in in_names
        ]
        concat_zeros = [
            np.zeros((B * av.shape[0], *av.shape[1:]), av.dtype)
            for av in out_avals
        ]
        out_arrs = sharded(*concat_in, *concat_zeros)
        return [
            {nm: np.asarray(out_arrs[i]).reshape(B, *out_avals[i].shape)[c]
             for i, nm in enumerate(out_names)}
            for c in range(B)
        ]

    return run


def kernel(**inputs):
    h_states = np.ascontiguousarray(np.asarray(inputs["h_states"], dtype=np.float32))
    W_heads = np.ascontiguousarray(np.asarray(inputs["W_heads"], dtype=np.float32))
    a_heads = np.ascontiguousarray(np.asarray(inputs["a_heads"], dtype=np.float32))
    W_out = np.ascontiguousarray(np.asarray(inputs["W_out"], dtype=np.float32))
    a_out = np.ascontiguousarray(np.asarray(inputs["a_out"], dtype=np.float32))

    if "nc" not in _NC_CACHE:
        _NC_CACHE["nc"] = build_kernel()
        _NC_CACHE["run"] = _make_runner(_NC_CACHE["nc"])

    xs = h_states.reshape(B, N, D)
    in_maps = [
        {"x": xs[c], "W_heads": W_heads, "a_heads": a_heads,
         "W_out": W_out, "a_out": a_out}
        for c in range(B)
    ]
    results = _NC_CACHE["run"](in_maps)
    return np.concatenate([results[c]["out"] for c in range(B)], axis=0)


if __name__ == "__main__":
    rng = np.random.default_rng(0)
    inputs = {
        "h_states": rng.standard_normal((B * N, D)).astype(np.float32),
        "W_heads": rng.standard_normal((H, D, D)).astype(np.float32) * 0.18,
        "a_heads": rng.standard_normal((H, 2 * D)).astype(np.float32) * 0.18,
        "W_out": rng.standard_normal((H * D, D)).astype(np.float32) * 0.09,
        "a_out": rng.standard_normal((2 * D,)).astype(np.float32) * 0.18,
        "seq_start_end": (np.arange(B, dtype=np.int32)[:, None] * N
                          + np.array([0, N], dtype=np.int32)[None, :]),
    }
    got = kernel(**inputs)
    print("kernel output", got.shape, got.dtype)


# revision 48
# speedup vs baseline: 1.0113x; 1.0074x over previous
"""Bass/Tile Trainium2 kernel for a 2-layer dense multi-head GAT over a batch
of B=8 independent subgraphs (2048 nodes each, equal contiguous segments).

Sharding: one subgraph per NeuronCore (8 cores), parameters replicated.

Algorithm (per core / subgraph, per attention layer):
  scores are rank-1:  e_ij = leaky_relu(s1_i + s2_j),  s1 = h@a1, s2 = h@a2.
  exp(leaky_relu(t)) is separable through the sign mask M_ij = [s1_i+s2_j>=0]:
      p_ij = M_ij e^{s1_i} e^{s2_j} + (1-M_ij) e^{a s1_i} e^{a s2_j}
  so softmax(e) @ h needs NO N^2 exp work:
      num_i = g_i (M @ u)_i + (vtot - (M @ v))_i        u_j = e^{s2_j} [h_j|1]
      out_i = num_i[:64] / num_i[64]                    v_j = e^{a s2_j}[h_j|1]
                                                        g_i = e^{(1-a) s1_i}
  The N^2 work is one compare pass (mask tiles, exact in bf16) plus bf16 mask
  matmuls.  Mask generation is split across three engines: DVE/GpSimd emit
  0/1 masks (tensor_scalar is_ge); ACT emits +-1 sign masks (Sign activation
  with per-partition bias).  Sign chunks stream uv at half scale so that
  (2M-1)@(u/2) = M@u - utot_c/2, folded back via the PSUM seed row:
      A = [sum_c Mc@u | sum_c Mc@v] + seed,  seed = [S_u | -(vtotA + vtotS)]
  making num = g*A_lo - A_hi uniformly.  Layer-1 elu outputs are stored as
  elu+1 (= relu(x)+exp(min(x,0))), removing one N-wide op per tile; the +1
  shift is corrected exactly downstream (log_softmax is shift-invariant; the
  layer-2 projections get constant corrections c = colsum(wa2), r = colsum(W)).
"""

from contextlib import ExitStack

import numpy as np

import concourse.bass as bass
import concourse.tile as tile
from concourse import bacc, mybir
from concourse.masks import make_identity

FP = mybir.dt.float32
BF = mybir.dt.bfloat16
AF = mybir.ActivationFunctionType
OP = mybir.AluOpType

B = 8
N = 2048
D = 64
H = 4
ALPHA = 0.2
P = 128
NCH = N // P  # 16 chunks of 128 nodes
DEXT = D + 1  # h plus ones column

# mask-generation engine per j-chunk (per layer): ACT emits sign masks,
# GpSimd and DVE emit 0/1 masks.  Spread so no engine's chunks cluster.
ACT_CHUNKS = (3, 7, 11, 14)
POOL_CHUNKS = (5, 15)
# the output layer's masks cannot pre-generate (they need all 4 heads), so
# shift more of them onto the otherwise-idle ACT/GpSimd engines there
L2_ACT_CHUNKS = ACT_CHUNKS
L2_POOL_CHUNKS = POOL_CHUNKS
# double-buffered mask tags (generated a layer ahead); the rest single-buffer
MASK_BUFS2 = frozenset(range(5))


def _seed_groups(L):
    grpA = [c for c in range(NCH) if c not in L.act_chunks]  # 0/1 chunks
    grpS = list(L.act_chunks)  # sign chunks
    return grpA, grpS


class _Layer:
    """Holds one attention layer's prep tensors."""

    def __init__(self, tag):
        self.tag = tag
        self.act_chunks = L2_ACT_CHUNKS if tag == "o" else ACT_CHUNKS
        self.pool_chunks = L2_POOL_CHUNKS if tag == "o" else POOL_CHUNKS
        self.s12 = None      # [P, NCH, 2] fp32 (s1|s2 per node chunk)
        self.s2m = None      # callable jc -> [P,1] scalar AP for the mask op
        self.s1b = None      # [P, N] bf16, s1 replicated along free dim
        self.hext = None     # [P, NCH, DEXT] bf16, col D == 1.0
        self.g = None        # [P, NCH] fp32
        self.es2 = None      # [P, NCH] fp32 e^{s2}
        self.nes02 = None    # [P, NCH] fp32 e^{a s2}
        self.es2h = None     # halved versions (for sign chunks)
        self.nes02h = None
        self.uv = None       # [P, NCH, 2*DEXT] bf16
        self.sd_hi = None    # [1, 2*DEXT] bf16 seed row (hi)
        self.sd_res = None   # [1, 2*DEXT] bf16 seed row (residual)
        self.masks = {}      # jc -> [P, N] mask tile


def _emit_exps_and_uv(nc, pools, scratch, L, corr=None):
    """exps (ACT), halved copies, uv tiles (DVE), vt matmuls + seed rows."""
    const, prep, maskp, wide, small, psA, psaux, psvt = pools
    s12 = L.s12

    es2 = prep.tile([P, NCH], FP, tag="es2", name=f"es2_{L.tag}")
    nes02 = prep.tile([P, NCH], FP, tag="nes02", name=f"nes02_{L.tag}")
    g = prep.tile([P, NCH], FP, tag="g", name=f"g_{L.tag}")
    if corr is None:
        nc.scalar.activation(es2, s12[:, :, 1], AF.Exp)
        nc.scalar.activation(nes02, s12[:, :, 1], AF.Exp, scale=ALPHA)
        nc.scalar.activation(g, s12[:, :, 0], AF.Exp, scale=1.0 - ALPHA)
    else:
        # corrections for the xc+1 shift: corr = [cs | -c2 | -a*c2 | -(1-a)c1]
        nc.scalar.activation(es2, s12[:, :, 1], AF.Exp, bias=corr[:, 1:2])
        nc.scalar.activation(nes02, s12[:, :, 1], AF.Exp, scale=ALPHA,
                             bias=corr[:, 2:3])
        nc.scalar.activation(g, s12[:, :, 0], AF.Exp, scale=1.0 - ALPHA,
                             bias=corr[:, 3:4])
    es2h = prep.tile([P, NCH], FP, tag="es2h", name=f"es2h_{L.tag}")
    nc.vector.tensor_scalar(es2h, es2, 0.5, None, OP.mult)
    nes02h = prep.tile([P, NCH], FP, tag="nes02h", name=f"nes02h_{L.tag}")
    nc.vector.tensor_scalar(nes02h, nes02, 0.5, None, OP.mult)
    L.g, L.es2, L.nes02, L.es2h, L.nes02h = g, es2, nes02, es2h, nes02h

    # uv tiles (bf16, 4x DVE mode); sign chunks use the halved scalars.
    # DVE/GpSimd split the per-chunk ops to balance engine load.
    uv = prep.tile([P, NCH, 2 * DEXT], BF, tag="uv", name=f"uv_{L.tag}")
    for c in range(NCH):
        eu, ev = (es2h, nes02h) if c in L.act_chunks else (es2, nes02)
        eng = nc.vector if c % 2 == 0 else nc.gpsimd
        eng.tensor_scalar(uv[:, c, 0:DEXT], L.hext[:, c, :],
                          eu[:, c:c + 1], None, OP.mult)
        eng.tensor_scalar(uv[:, c, DEXT:], L.hext[:, c, :],
                          ev[:, c:c + 1], None, OP.mult)
    L.uv = uv


def _emit_vt_seed(nc, pools, scratch, L):
    """Column-total matmuls + seed rows.  Emitted late (hook q2) so the PE
    queue position comes after work whose inputs are long-ready -- the vt
    matmuls need ALL 16 uv chunks and would head-of-line-block the PE."""
    const, prep, maskp, wide, small, psA, psaux, psvt = pools
    uv = L.uv
    # column totals: vtA over 0/1 chunks (full scale), vtS over sign chunks
    # (half scale, exactly the S_c/2 sums the seed needs)
    grpA, grpS = _seed_groups(L)
    ones_col_bf = scratch["ones_col_bf"]
    vtA = psvt.tile([1, 2 * DEXT], FP, tag="vt", name=f"vtA_{L.tag}")
    vtS = psvt.tile([1, 2 * DEXT], FP, tag="vt", name=f"vtS_{L.tag}")
    for i, c in enumerate(grpA):
        nc.tensor.matmul(vtA, ones_col_bf, uv[:, c, :], start=(i == 0),
                         stop=(i == len(grpA) - 1))
    for i, c in enumerate(grpS):
        nc.tensor.matmul(vtS, ones_col_bf, uv[:, c, :], start=(i == 0),
                         stop=(i == len(grpS) - 1))

    # seed row sd = [vtS_u | -(vtA_v + vtS_v)] in fp32, then bf16 hi+res
    vts_sb = prep.tile([1, 2 * DEXT], FP, tag="vts", name=f"vts_{L.tag}")
    nc.vector.tensor_copy(vts_sb, vtS)
    sd = prep.tile([1, 2 * DEXT], FP, tag="sd", name=f"sd_{L.tag}")
    nc.vector.tensor_copy(sd[:, 0:DEXT], vts_sb[:, 0:DEXT])
    nc.vector.scalar_tensor_tensor(sd[:, DEXT:], vtA[:, DEXT:], -1.0,
                                   vts_sb[:, DEXT:], OP.mult, OP.subtract)
    sd_hi = prep.tile([1, 2 * DEXT], BF, tag="sdh", name=f"sdh_{L.tag}")
    nc.vector.tensor_copy(sd_hi, sd)
    sd_res = prep.tile([1, 2 * DEXT], BF, tag="sdr", name=f"sdr_{L.tag}")
    nc.vector.tensor_tensor(sd_res, sd, sd_hi, OP.subtract)
    L.sd_hi, L.sd_res = sd_hi, sd_res


def _emit_mask(nc, pools, L, jc):
    """One full-i mask tile [P, N] for j-chunk jc, on its assigned engine."""
    const, prep, maskp, wide, small, psA, psaux, psvt = pools
    bufs = 2 if jc in MASK_BUFS2 else 1
    mt = maskp.tile([P, N], BF, tag=f"m{jc}", name=f"m{jc}_{L.tag}",
                    bufs=bufs)
    s2s = L.s2m(jc)
    if jc in L.act_chunks:
        nc.scalar.activation(mt, L.s1b, AF.Sign, bias=s2s)
    elif jc in L.pool_chunks:
        nc.gpsimd.tensor_scalar(mt, L.s1b, s2s, 0.0, OP.add, OP.is_ge)
    else:
        nc.vector.tensor_scalar(mt, L.s1b, s2s, 0.0, OP.add, OP.is_ge)
    L.masks[jc] = mt


def _attention_quarters(nc, pools, scratch, L, out_cb, hooks=None):
    """Mask matmuls + epilogue for the 4 quarters of the i axis.

    hooks: optional dict q -> callable, emitted after quarter q's
    instructions (used to interleave the next layer's prep/mask emission).
    """
    const, prep, maskp, wide, small, psA, psaux, psvt = pools
    ones_row_bf = scratch["ones_row_bf"]

    nsum = wide.tile([P, NCH, DEXT], FP, tag="nsum", name=f"nsum_{L.tag}")
    rz = wide.tile([P, NCH], FP, tag="rz", name=f"rz_{L.tag}")
    onorm = wide.tile([P, NCH, D], FP, tag="onorm", name=f"onorm_{L.tag}")

    for q in range(4):
        # two accumulators share one PSUM bank -> bufs=4 spans two quarters,
        # so quarter q+1's matmuls never wait on quarter q's evacuation
        Ap = [psA.tile([P, 2, 2 * DEXT], FP, tag="A",
                       name=f"A{L.tag}_{q}_{pi}") for pi in range(2)]
        A = [Ap[il // 2][:, il % 2, :] for il in range(4)]
        for jc in range(NCH):
            mt = L.masks[jc]
            for il in range(4):
                sl = mt[:, q * 512 + il * P: q * 512 + (il + 1) * P]
                # start=True zeroes the whole 2KB bank, so only the pair's
                # first region sets it; the partner lands on zeroed PSUM
                nc.tensor.matmul(A[il], sl, L.uv[:, jc, :],
                                 start=(jc == 0 and il % 2 == 0), stop=False,
                                 skip_group_check=(il % 2 == 1))
        for il in range(4):
            nc.tensor.matmul(A[il], ones_row_bf, L.sd_hi, start=False,
                             stop=False)
            nc.tensor.matmul(A[il], ones_row_bf, L.sd_res, start=False,
                             stop=True)
        # epilogue: ACT evacuates each pair in one op, DVE combines
        # num = g*A_lo - A_hi from SBUF
        for pi in range(2):
            Asb = small.tile([P, 2, 2 * DEXT], FP, tag="Asb",
                             name=f"Asb_{L.tag}_{q}_{pi}")
            nc.scalar.copy(Asb, Ap[pi])
            for k in range(2):
                ic = q * 4 + pi * 2 + k
                nc.vector.scalar_tensor_tensor(nsum[:, ic, :],
                                               Asb[:, k, 0:DEXT],
                                               L.g[:, ic:ic + 1],
                                               Asb[:, k, DEXT:],
                                               OP.mult, OP.subtract)
        qs = slice(q * 4, (q + 1) * 4)
        nc.vector.reciprocal(rz[:, qs], nsum[:, qs, D])
        rzq = rz[:, qs]
        rz_b = bass.AP(tensor=rzq.tensor, offset=rzq.offset,
                       ap=[rzq.ap[0], rzq.ap[1], [0, D]])
        nc.vector.tensor_tensor(onorm[:, qs, :], nsum[:, qs, 0:D], rz_b,
                                OP.mult)
        out_cb(onorm, q)
        if hooks and q in hooks:
            hooks[q]()
    L.masks = {}


def _attention_halves(nc, pools, scratch, L, out_cb):
    """jc-major variant: 8 accumulators span an i-half; the j-chunk loop
    streams all 16 masks, consuming each in 8 matmuls (~450ns) -- slower than
    the 3-engine mask production rate, so a layer whose masks cannot
    pre-generate (the output layer) never stalls PE on mask availability."""
    const, prep, maskp, wide, small, psA, psaux, psvt = pools
    ones_row_bf = scratch["ones_row_bf"]

    nsum = wide.tile([P, NCH, DEXT], FP, tag="nsum", name=f"nsum_{L.tag}")
    rz = wide.tile([P, NCH], FP, tag="rz", name=f"rz_{L.tag}")
    onorm = wide.tile([P, NCH, D], FP, tag="onorm", name=f"onorm_{L.tag}")

    for hf in range(2):
        Ap = [psA.tile([P, 2, 2 * DEXT], FP, tag="A",
                       name=f"A{L.tag}_{hf}_{pi}") for pi in range(4)]
        A = [Ap[il // 2][:, il % 2, :] for il in range(8)]
        for jc in range(NCH):
            mt = L.masks[jc]
            for il in range(8):
                sl = mt[:, hf * 1024 + il * P: hf * 1024 + (il + 1) * P]
                nc.tensor.matmul(A[il], sl, L.uv[:, jc, :],
                                 start=(jc == 0 and il % 2 == 0), stop=False,
                                 skip_group_check=(il % 2 == 1))
        for il in range(8):
            nc.tensor.matmul(A[il], ones_row_bf, L.sd_hi, start=False,
                             stop=False)
            nc.tensor.matmul(A[il], ones_row_bf, L.sd_res, start=False,
                             stop=True)
        for pi in range(4):
            Asb = small.tile([P, 2, 2 * DEXT], FP, tag="Asb",
                             name=f"Asb_{L.tag}_{hf}_{pi}")
            nc.scalar.copy(Asb, Ap[pi])
            for k in range(2):
                ic = hf * 8 + pi * 2 + k
                nc.vector.scalar_tensor_tensor(nsum[:, ic, :],
                                               Asb[:, k, 0:DEXT],
                                               L.g[:, ic:ic + 1],
                                               Asb[:, k, DEXT:],
                                               OP.mult, OP.subtract)
        for sq in range(2):
            qs = slice((hf * 2 + sq) * 4, (hf * 2 + sq + 1) * 4)
            nc.vector.reciprocal(rz[:, qs], nsum[:, qs, D])
            rzq = rz[:, qs]
            rz_b = bass.AP(tensor=rzq.tensor, offset=rzq.offset,
                           ap=[rzq.ap[0], rzq.ap[1], [0, D]])
            nc.vector.tensor_tensor(onorm[:, qs, :], nsum[:, qs, 0:D], rz_b,
                                    OP.mult)
        out_cb(onorm, hf)
    L.masks = {}


def _elu1_q(nc, wide, onorm, q, tag, dst, dst_sl, dve=False):
    """elu+1 over quarter q of onorm: dst = max(o,0) + exp(min(o,0))."""
    src = onorm[:, q * 4:(q + 1) * 4, :]
    eng = nc.vector if dve else nc.gpsimd
    m = wide.tile([P, 4, D], FP, tag="elu_m", name=f"elu_m{tag}_{q}")
    eng.tensor_scalar(m, src, 0.0, None, OP.min)
    e = wide.tile([P, 4, D], FP, tag="elu_e", name=f"elu_e{tag}_{q}")
    nc.scalar.activation(e, m, AF.Exp)
    r = wide.tile([P, 4, D], FP, tag="elu_r", name=f"elu_r{tag}_{q}")
    eng.tensor_scalar(r, src, 0.0, None, OP.max)
    eng.tensor_tensor(dst[dst_sl], r, e, OP.add)


def build_kernel():
    nc = bacc.Bacc("TRN2", target_bir_lowering=False, debug=False,
                   num_devices=B)

    x = nc.dram_tensor("x", [N, D], FP, kind="ExternalInput")
    W_heads = nc.dram_tensor("W_heads", [H, D, D], FP, kind="ExternalInput")
    a_heads = nc.dram_tensor("a_heads", [H, 2 * D], FP, kind="ExternalInput")
    W_out = nc.dram_tensor("W_out", [H * D, D], FP, kind="ExternalInput")
    a_out = nc.dram_tensor("a_out", [2 * D], FP, kind="ExternalInput")
    out = nc.dram_tensor("out", [N, D], FP, kind="ExternalOutput")

    with tile.TileContext(nc) as tc, ExitStack() as ctx:
        const = ctx.enter_context(tc.tile_pool(name="const", bufs=1))
        prep = ctx.enter_context(tc.tile_pool(name="prep", bufs=2))
        maskp = ctx.enter_context(tc.tile_pool(name="maskp", bufs=1))
        wide = ctx.enter_context(tc.tile_pool(name="wide", bufs=2))
        small = ctx.enter_context(tc.tile_pool(name="small", bufs=4))
        psA = ctx.enter_context(tc.tile_pool(name="psA", bufs=4, space="PSUM"))
        psaux = ctx.enter_context(tc.tile_pool(name="psaux", bufs=2,
                                               space="PSUM"))
        psvt = ctx.enter_context(tc.tile_pool(name="psvt", bufs=2,
                                              space="PSUM"))
        pools = (const, prep, maskp, wide, small, psA, psaux, psvt)

        ident = const.tile([P, P], FP)
        make_identity(nc, ident)
        ones128 = const.tile([P, P], FP)
        nc.vector.memset(ones128, 1.0)
        ones_col_bf = const.tile([P, 1], BF)
        nc.vector.memset(ones_col_bf, 1.0)
        ones_row_bf = const.tile([1, P], BF)
        nc.vector.memset(ones_row_bf, 1.0)
        scratch = {"ones128": ones128, "ones_col_bf": ones_col_bf,
                   "ones_row_bf": ones_row_bf}

        # ---- input DMAs: x in 4 pieces, params in NATURAL layouts only
        # (transposed layouts would be 4-byte-gather DMAs; transpose on PE).
        # Interleaved so HWDGE serializing 632ns/DMA doesn't delay the params
        # (which gate the wa/s12 chain) behind all of x.
        x_sb = const.tile([P, NCH, D], FP)
        x_r = x.rearrange("(c p) d -> p c d", p=P)
        Wh = const.tile([64, H, D], FP)
        a_sb = const.tile([64, H, 2], FP)
        Wo = const.tile([P, 2, D], FP)
        ao = const.tile([64, 2], FP)

        def xdma(r4):
            nc.sync.dma_start(out=x_sb[:, r4 * 4:(r4 + 1) * 4, :],
                              in_=x_r[:, r4 * 4:(r4 + 1) * 4, :])
        xdma(0)
        nc.sync.dma_start(out=Wh, in_=W_heads.rearrange("h k d -> k h d"))
        nc.sync.dma_start(out=a_sb,
                          in_=a_heads.rearrange("h (t k) -> k h t", t=2))
        xdma(1)
        nc.sync.dma_start(out=Wo, in_=W_out.rearrange("(c k) d -> k c d", k=P))
        nc.sync.dma_start(out=ao, in_=a_out.rearrange("(t k) -> k t", t=2))
        xdma(2)
        xdma(3)

        # ---- param transposes on PE + bf16 shadows ----
        WhT = const.tile([64, H, D], FP)
        for h in range(H):
            tp = psaux.tile([64, D], FP, tag="aux", name=f"whT{h}")
            nc.tensor.transpose(tp, Wh[:, h, :], ident[0:64, 0:64])
            nc.scalar.copy(WhT[:, h, :], tp)
        WoT = const.tile([64, 2, P], FP)
        for kc in range(2):
            tp = psaux.tile([64, P], FP, tag="aux", name=f"woT{kc}")
            nc.tensor.transpose(tp, Wo[:, kc, :], ident)
            nc.scalar.copy(WoT[:, kc, :], tp)
        Wh_bf = const.tile([64, H, D], BF)
        nc.vector.tensor_copy(Wh_bf, Wh)
        Wo_bf = const.tile([P, 2, D], BF)
        nc.vector.tensor_copy(Wo_bf, Wo)

        # ---- x transposes -> xT fp32 + bf16 shadow ----
        xT = const.tile([64, N], FP)
        for c in range(NCH):
            tp = psaux.tile([64, P], FP, tag="aux", name=f"xT{c}")
            nc.tensor.transpose(tp, x_sb[:, c, :], ident)
            if c % 2 == 0:
                nc.vector.tensor_copy(xT[:, c * P:(c + 1) * P], tp)
            else:
                nc.scalar.copy(xT[:, c * P:(c + 1) * P], tp)
        xT_bf = const.tile([64, N], BF)
        for r in range(4):
            eng = nc.vector if r % 2 == 0 else nc.gpsimd
            eng.tensor_copy(xT_bf[:, r * 512:(r + 1) * 512],
                            xT[:, r * 512:(r + 1) * 512])

        # all heads' wa = W_h @ [a1|a2] upfront
        wa_all = const.tile([64, H, 2], FP)
        for h in range(H):
            wap = psaux.tile([64, 2], FP, tag="aux", name=f"wap{h}")
            nc.tensor.matmul(wap, WhT[:, h, :], a_sb[:, h, :], start=True,
                             stop=True)
            nc.scalar.copy(wa_all[:, h, :], wap)

        # ================= layer 1: four heads =================
        xc01 = const.tile([P, NCH, 2, D], FP)
        xc23 = const.tile([P, NCH, 2, D], FP)

        def l1_prep(h):
            L = _Layer(f"h{h}")
            wa = wa_all[:, h, :]
            s12 = prep.tile([P, NCH, 2], FP, tag="s12", name=f"s12_{h}")
            for cg in range(4):
                sp = psaux.tile([P, 8], FP, tag="aux", name=f"sp{h}_{cg}")
                for k in range(4):
                    c = cg * 4 + k
                    nc.tensor.matmul(sp[:, 2 * k:2 * k + 2],
                                     xT[:, c * P:(c + 1) * P], wa,
                                     start=True, stop=True)
                nc.vector.tensor_copy(s12[:, cg * 4:(cg + 1) * 4, :], sp)
            L.s12 = s12
            L.s2m = lambda jc: s12[:, jc, 1:2]

            wa1b = prep.tile([64, P], BF, tag="wa1b", name=f"wa1b_{h}")
            nc.vector.tensor_scalar(wa1b, ones128[0:64, :], wa[:, 0:1], None,
                                    OP.mult)
            s1b = prep.tile([P, N], BF, tag="s1b", name=f"s1b_{h}")
            for r in range(4):
                ps = psaux.tile([P, 512], FP, tag="aux", name=f"s1bp{h}_{r}")
                nc.tensor.matmul(ps, wa1b, xT_bf[:, r * 512:(r + 1) * 512],
                                 start=True, stop=True)
                nc.scalar.copy(s1b[:, r * 512:(r + 1) * 512], ps)
            L.s1b = s1b

            hext = prep.tile([P, NCH, DEXT], BF, tag="hext", name=f"hext_{h}")
            nc.vector.memset(hext[:, :, D], 1.0)
            for c in range(NCH):
                hp = psaux.tile([P, D], FP, tag="aux", name=f"hp{h}_{c}")
                nc.tensor.matmul(hp, xT_bf[:, c * P:(c + 1) * P],
                                 Wh_bf[:, h, :], start=True, stop=True)
                nc.vector.tensor_copy(hext[:, c, 0:D], hp)
            L.hext = hext
            _emit_exps_and_uv(nc, pools, scratch, L)
            return L

        def l1_out(L, h):
            xc = xc01 if h < 2 else xc23

            def cb(onorm, q):
                _elu1_q(nc, wide, onorm, q, f"h{h}", xc,
                        np.s_[:, q * 4:(q + 1) * 4, h % 2, :])
            return cb

        # ---- xc transposes (emitted interleaved via hooks) ----
        xcT = const.tile([P, 2, N], FP)
        xcT_bf = const.tile([P, 2, N], BF)

        def xc_transpose(kc, c0, c1):
            xc = xc01 if kc == 0 else xc23
            for c in range(c0, c1):
                tp = psaux.tile([P, P], FP, tag="aux", name=f"xcT{kc}_{c}")
                nc.tensor.transpose(tp, xc[:, c, :, :], ident)
                if c % 2 == 0:
                    nc.vector.tensor_copy(xcT[:, kc, c * P:(c + 1) * P], tp)
                else:
                    nc.scalar.copy(xcT[:, kc, c * P:(c + 1) * P], tp)

        def xcbf_copy(kc, r0, r1):
            for r in range(r0, r1):
                eng = nc.gpsimd if r % 2 == 0 else nc.vector
                eng.tensor_copy(xcT_bf[:, kc, r * 512:(r + 1) * 512],
                                xcT[:, kc, r * 512:(r + 1) * 512])

        # ================= layer 2 prep =================
        def l2_prep_part1():
            """Everything that only needs parameters (+ c/r correction rows)."""
            st = {}
            wa2 = prep.tile([P, 2, 2], FP, tag="wa2")
            for kc in range(2):
                wap = psaux.tile([P, 2], FP, tag="aux", name=f"wap2_{kc}")
                nc.tensor.matmul(wap, WoT[:, kc, :], ao, start=True, stop=True)
                nc.scalar.copy(wa2[:, kc, :], wap)
            st["wa2"] = wa2
            # c = colsum(wa2) [1,2] -> broadcast [128,2]; r = colsum(W_out)
            c_ps = psvt.tile([1, 2], FP, tag="vt", name="c_ps")
            for kc in range(2):
                nc.tensor.matmul(c_ps, ones128[:, 0:1], wa2[:, kc, :],
                                 start=(kc == 0), stop=(kc == 1))
            c_sb = prep.tile([1, 2], FP, tag="c_sb")
            nc.vector.tensor_copy(c_sb, c_ps)
            cb_ps = psvt.tile([P, 2], FP, tag="vt", name="cb_ps")
            nc.tensor.matmul(cb_ps, ones128[0:1, :], c_sb, start=True,
                             stop=True)
            cb = prep.tile([P, 2], FP, tag="cb")
            nc.vector.tensor_copy(cb, cb_ps)
            corr = prep.tile([P, 4], FP, tag="corr")
            nc.vector.tensor_tensor(corr[:, 0:1], cb[:, 0:1], cb[:, 1:2],
                                    OP.add)
            nc.vector.tensor_scalar(corr[:, 1:2], cb[:, 1:2], -1.0, None,
                                    OP.mult)
            nc.vector.tensor_scalar(corr[:, 2:3], cb[:, 1:2], -ALPHA, None,
                                    OP.mult)
            nc.vector.tensor_scalar(corr[:, 3:4], cb[:, 0:1],
                                    -(1.0 - ALPHA), None, OP.mult)
            st["corr"] = corr
            r_ps = psvt.tile([1, D], FP, tag="vt", name="r_ps")
            for kc in range(2):
                nc.tensor.matmul(r_ps, ones_col_bf, Wo_bf[:, kc, :],
                                 start=(kc == 0), stop=(kc == 1))
            nr = prep.tile([1, D], BF, tag="nr")
            nc.vector.tensor_scalar(nr, r_ps, -1.0, None, OP.mult)
            st["nr"] = nr
            return st

        def l2_prep_part2(st):
            """Needs xcT/xcT_bf: s12_2, s2m, s1b_2, hext2, exps, uv, seeds."""
            L = _Layer("o")
            wa2, corr, nr = st["wa2"], st["corr"], st["nr"]
            s12 = prep.tile([P, NCH, 2], FP, tag="s12", name="s12_o")
            for cg in range(4):
                sp = psaux.tile([P, 8], FP, tag="aux", name=f"sp2_{cg}")
                for k in range(4):
                    c = cg * 4 + k
                    for kc in range(2):
                        nc.tensor.matmul(sp[:, 2 * k:2 * k + 2],
                                         xcT[:, kc, c * P:(c + 1) * P],
                                         wa2[:, kc, :],
                                         start=(kc == 0), stop=(kc == 1))
                nc.vector.tensor_copy(s12[:, cg * 4:(cg + 1) * 4, :], sp)
            L.s12 = s12
            # s2m = s2' - (c1+c2) so the masks compare the true s1+s2 >= 0
            s2m = prep.tile([P, NCH], FP, tag="s2m")
            nc.vector.tensor_scalar(s2m, s12[:, :, 1], corr[:, 0:1], None,
                                    OP.subtract)
            L.s2m = lambda jc: s2m[:, jc:jc + 1]

            wa1b2 = prep.tile([P, 2, P], BF, tag="wa1b2")
            for kc in range(2):
                nc.vector.tensor_scalar(wa1b2[:, kc, :], ones128,
                                        wa2[:, kc, 0:1], None, OP.mult)
            s1b = prep.tile([P, N], BF, tag="s1b", name="s1b_o")
            for r in range(4):
                ps = psaux.tile([P, 512], FP, tag="aux", name=f"s1bp_o{r}")
                for kc in range(2):
                    nc.tensor.matmul(ps, wa1b2[:, kc, :],
                                     xcT_bf[:, kc, r * 512:(r + 1) * 512],
                                     start=(kc == 0), stop=(kc == 1))
                nc.scalar.copy(s1b[:, r * 512:(r + 1) * 512], ps)
            L.s1b = s1b

            hext = prep.tile([P, NCH, DEXT], BF, tag="hext", name="hext_o")
            nc.vector.memset(hext[:, :, D], 1.0)
            for c in range(NCH):
                hp = psaux.tile([P, D], FP, tag="aux", name=f"hp_o{c}")
                for kc in range(2):
                    nc.tensor.matmul(hp, xcT_bf[:, kc, c * P:(c + 1) * P],
                                     Wo_bf[:, kc, :], start=(kc == 0),
                                     stop=False)
                nc.tensor.matmul(hp, ones_row_bf, nr, start=False, stop=True)
                nc.vector.tensor_copy(hext[:, c, 0:D], hp)
            L.hext = hext
            _emit_exps_and_uv(nc, pools, scratch, L, corr=corr)
            return L

        # ================= emission schedule =================
        Ls = [None] * 5  # heads 0..3 + output layer
        Ls[0] = l1_prep(0)
        _emit_vt_seed(nc, pools, scratch, Ls[0])
        for jc in range(NCH):
            _emit_mask(nc, pools, Ls[0], jc)

        l2st = {}

        def mk_hooks(h):
            # Interleave next layer's prep + mask emission into head h's
            # quarter stream.  Only double-buffered mask tags may be emitted
            # before head h's last quarter (single-buffered tags would be
            # overwritten before head h's q2/q3 matmuls are even emitted).
            def prep_next():
                if h + 1 <= 3:
                    Ls[h + 1] = l1_prep(h + 1)
                    _emit_vt_seed(nc, pools, scratch, Ls[h + 1])
                if h == 2:
                    xc_transpose(0, 12, NCH)

            def masks_a():
                if h + 1 <= 3:
                    for jc in sorted(MASK_BUFS2):
                        _emit_mask(nc, pools, Ls[h + 1], jc)
                if h == 1:
                    xc_transpose(0, 0, 8)
                if h == 2:
                    xcbf_copy(0, 0, 4)

            def masks_b():
                if h == 1:
                    xc_transpose(0, 8, 12)
                if h == 2:
                    l2st.update(l2_prep_part1())

            def masks_c():
                if h + 1 <= 3:
                    for jc in range(NCH):
                        if jc not in MASK_BUFS2:
                            _emit_mask(nc, pools, Ls[h + 1], jc)
            return {0: prep_next, 1: masks_a, 2: masks_b, 3: masks_c}

        for h in range(H):
            hooks = mk_hooks(h)
            if h == 3:
                base = hooks
                def h3_hooks(q, base=base):
                    def f():
                        if q in base:
                            base[q]()
                        if q >= 1:
                            xc_transpose(1, (q - 1) * 4, q * 4)
                    return f
                hooks = {q: h3_hooks(q) for q in range(4)}
            _attention_quarters(nc, pools, scratch, Ls[h], l1_out(Ls[h], h),
                                hooks)

        xc_transpose(1, 12, NCH)
        xcbf_copy(1, 0, 4)
        Ls[4] = l2_prep_part2(l2st)
        _emit_vt_seed(nc, pools, scratch, Ls[4])
        for jc in range(NCH):
            _emit_mask(nc, pools, Ls[4], jc)

        # ================= layer 2 attention + log_softmax =================
        o2_all = const.tile([P, NCH, D], FP)
        esum = const.tile([P, NCH], FP)
        lse = const.tile([P, NCH], FP)
        out_r = out.rearrange("(c p) d -> p c d", p=P)

        def finish_half(hf):
            # Ln batched per half: Exp and Ln live in different default act
            # tables, so finer batching would thrash LoadActFuncSet
            sl = slice(hf * 8, (hf + 1) * 8)
            nc.scalar.activation(lse[:, sl], esum[:, sl], AF.Ln)
            for sq in range(2):
                qs = slice(hf * 8 + sq * 4, hf * 8 + (sq + 1) * 4)
                lseh = lse[:, qs]
                lse_b = bass.AP(tensor=lseh.tensor, offset=lseh.offset,
                                ap=[lseh.ap[0], lseh.ap[1], [0, D]])
                nc.vector.tensor_tensor(o2_all[:, qs, :], o2_all[:, qs, :],
                                        lse_b, OP.subtract)
                nc.sync.dma_start(out=out_r[:, qs, :], in_=o2_all[:, qs, :])

        def l2_out(onorm, hf):
            for sq in range(2):
                q = hf * 2 + sq
                qs = slice(q * 4, (q + 1) * 4)
                _elu1_q(nc, wide, onorm, q, "o", o2_all, np.s_[:, qs, :])
                escr = wide.tile([P, 4, D], FP, tag="escr", name=f"escr{q}")
                nc.scalar.activation(escr, o2_all[:, qs, :], AF.Exp)
                nc.vector.tensor_reduce(esum[:, qs], escr,
                                        mybir.AxisListType.X, OP.add)
            finish_half(hf)

        _attention_halves(nc, pools, scratch, Ls[4], l2_out)

    nc.compile()
    return nc


_NC_CACHE = {}


def _make_runner(nc):
    """Build a cached sharded executable (run_bass_kernel_spmd re-traces
    jax.jit on every call; this jits once and reuses)."""
    import jax
    from jax.sharding import Mesh, PartitionSpec
    try:
        from jax.experimental.shard_map import shard_map
    except ImportError:
        from jax.shard_map import shard_map
    import concourse.mybir as mb
    from concourse import bass2jax

    bass2jax.install_neuronx_cc_hook()

    part_name = nc.partition_id_tensor.name if nc.partition_id_tensor else None
    in_names, out_names, out_avals = [], [], []
    for alloc in nc.m.functions[0].allocations:
        if not isinstance(alloc, mb.MemoryLocationSet):
            continue
        name = alloc.memorylocations[0].name
        if alloc.kind == "ExternalInput":
            if name != part_name:
                in_names.append(name)
        elif alloc.kind == "ExternalOutput":
            out_names.append(name)
            out_avals.append(jax.core.ShapedArray(
                tuple(alloc.tensor_shape), mb.dt.np(alloc.dtype)))
    n_params = len(in_names)
    all_names = in_names + out_names
    if part_name is not None:
        all_names = all_names + [part_name]

    def _body(*args):
        operands = list(args)
        if part_name is not None:
            operands.append(bass2jax.partition_id_tensor())
        return tuple(bass2jax._bass_exec_p.bind(
            *operands, out_avals=tuple(out_avals), in_names=tuple(all_names),
            out_names=tuple(out_names), lowering_input_output_aliases=(),
            sim_require_finite=True, sim_require_nnan=True, nc=nc))

    devices = jax.devices()[:B]
    mesh = Mesh(np.asarray(devices), ("core",))
    n_outs = len(out_names)
    sharded = jax.jit(
        shard_map(_body, mesh=mesh,
                  in_specs=(PartitionSpec("core"),) * (n_params + n_outs),
                  out_specs=(PartitionSpec("core"),) * n_outs,
                  check_rep=False),
        donate_argnums=tuple(range(n_params, n_params + n_outs)),
        keep_unused=True)

    def run(in_maps):
        concat_in = [
            np.concatenate([np.asarray(in_maps[c][nm])[None] for c in range(B)],
                           axis=0).reshape(B * in_maps[0][nm].shape[0],
                                           *in_maps[0][nm].shape[1:])
            for nm in in_names
        ]
        concat_zeros = [
            np.zeros((B * av.shape[0], *av.shape[1:]), av.dtype)
            for av in out_avals
        ]
        out_arrs = sharded(*concat_in, *concat_zeros)
        return [
            {nm: np.asarray(out_arrs[i]).reshape(B, *out_avals[i].shape)[c]
             for i, nm in enumerate(out_names)}
            for c in range(B)
        ]

    return run


def kernel(**inputs):
    h_states = np.ascontiguousarray(np.asarray(inputs["h_states"], dtype=np.float32))
    W_heads = np.ascontiguousarray(np.asarray(inputs["W_heads"], dtype=np.float32))
    a_heads = np.ascontiguousarray(np.asarray(inputs["a_heads"], dtype=np.float32))
    W_out = np.ascontiguousarray(np.asarray(inputs["W_out"], dtype=np.float32))
    a_out = np.ascontiguousarray(np.asarray(inputs["a_out"], dtype=np.float32))

    if "nc" not in _NC_CACHE:
        _NC_CACHE["nc"] = build_kernel()
        _NC_CACHE["run"] = _make_runner(_NC_CACHE["nc"])

    xs = h_states.reshape(B, N, D)
    in_maps = [
        {"x": xs[c], "W_heads": W_heads, "a_heads": a_heads,
         "W_out": W_out, "a_out": a_out}
        for c in range(B)
    ]
    results = _NC_CACHE["run"](in_maps)
    return np.concatenate([results[c]["out"] for c in range(B)], axis=0)


if __name__ == "__main__":
    rng = np.random.default_rng(0)
    inputs = {
        "h_states": rng.standard_normal((B * N, D)).astype(np.float32),
        "W_heads": rng.standard_normal((H, D, D)).astype(np.float32) * 0.18,
        "a_heads": rng.standard_normal((H, 2 * D)).astype(np.float32) * 0.18,
        "W_out": rng.standard_normal((H * D, D)).astype(np.float32) * 0.09,
        "a_out": rng.standard_normal((2 * D,)).astype(np.float32) * 0.18,
        "seq_start_end": (np.arange(B, dtype=np.int32)[:, None] * N
                          + np.array([0, N], dtype=np.int32)[None, :]),
    }
    got = kernel(**inputs)
    print("kernel output", got.shape, got.dtype)
